# revision 38
# baseline (speedup 1.0000x reference)
"""Trainium2 Bass kernel for nn_EncoderLayer (GNN message passing, 2-relation GAT).

Sharding: nodes (and incoming-edge lists, partitioned by dst) sharded across 8
cores; small GAT/FFN weights replicated; gathered src features fetched from a
replicated projection table via indexed DMA (dma_gather).

v4 layout (per core, node ids ROTATED so own chunk = rows [0, CHUNK)):
  Phase 0: fold weights (fp8 wpair with er columns appended); stage gather
           indices.
  Phase 1: BN1 (bf16 x input, vector rsqrt poly+Newton) + z/el projection for
           ALL N nodes via fp8 matmuls; packed rows
           zpackB[2*rot(node) + rel] = [768 z fp8 | 96B el bf16 | pad] (1024B).
           Blocks 0..9 are this core's own dst windows: er columns (cols
           272:288 of the same matmul) are stashed on-chip in erw_all.
  Phase 2 (per dst-window): gather (plain, 1024B fp8 rows, PREPD-deep ring);
           er broadcast edge-wise via ST fp8 matmul; lk = el(bf16 view) + ebc;
           exp(lk) written fp8 IN-PLACE into the gathered rows' el slot;
           mz = ex (x) z in place (fp8); segment-sum via fp8 one-hot S matmuls
           (816 contiguous fp8 cols = 768 msg + 48 denom); m = msgsum/denom;
           x2 = x + m1 + m2; BN2; FFN interleaved in the same window.
"""

import sys

sys.path.insert(0, "/opt/trn_rl_repo")

import numpy as np
import ml_dtypes

import concourse.bass as bass
import concourse.bacc as bacc
import concourse.tile as tile
import concourse.mybir as mybir
from concourse.bass_utils import run_bass_kernel_spmd

F32 = mybir.dt.float32
BF16 = mybir.dt.bfloat16
FP8 = mybir.dt.float8e4
I16 = mybir.dt.int16
AF = mybir.ActivationFunctionType
ALU = mybir.AluOpType
BF16NP = ml_dtypes.bfloat16
F8NP = ml_dtypes.float8_e4m3

N, T, D, H, DH, DFF = 10000, 12, 64, 4, 16, 128
NCORES = 8
CHUNK = N // NCORES          # 1250
WIN = 128                    # dst-window size (nodes)
NW = (CHUNK + WIN - 1) // WIN  # 10 windows; last has 98 nodes
EPS = 1e-5
NEG_SLOPE = 0.2
ZROW = 896                   # zpack row (bf16 elems): 12*68 data + 80 pad
NBLK = (N + 127) // 128      # 79 phase-1 blocks (last = 16 nodes)
NPAIR = T // 2               # 6 paired (2-timestep) transposes per block
SUP = 8                      # phase-1 super-block (batched DMA + rsqrt math)
PREPD = 3                    # gather ring depth (zg bufs)

# rsqrt(v + EPS) = quadratic fit + one Newton step (vector engine only).
_BN1_RANGE = (0.55, 1.6)
_BN2_RANGE = (0.55, 3.2)


def _rsqrt_coeffs(lo, hi):
    v = np.linspace(lo, hi, 4001)
    c = np.polyfit(v, 1.0 / np.sqrt(v + EPS), 2)
    return [float(x) for x in c]


def _win_nodes(w):
    return min(WIN, CHUNK - w * WIN)


def _prep_core_rel(src, dst, lo, B):
    """Edge lists for one (core, relation): sorted by dst, windowed, padded
    to B blocks of 128 edges per window. Node ids are ROTATED by -lo mod N.
    Returns (src_flat[NW*B*128], S fp8, ST fp8)."""
    hi = lo + CHUNK
    sel = (dst >= lo) & (dst < hi)
    es = ((src[sel].astype(np.int64) - lo) % N)    # rotated src ids
    ed = (dst[sel] - lo).astype(np.int64)
    order = np.argsort(ed, kind="stable")
    es, ed = es[order], ed[order]
    L = NW * B * 128
    src_arr = np.zeros(L, np.int64)
    S = np.zeros((NW, 128, B * 128), F8NP)
    ST = np.zeros((NW, 128, B * 128), F8NP)
    wstart = np.searchsorted(ed, np.arange(NW) * WIN)
    wend = np.searchsorted(ed, np.arange(1, NW + 1) * WIN)
    for w in range(NW):
        seg_src = es[wstart[w]:wend[w]]
        seg_dst = ed[wstart[w]:wend[w]] - w * WIN
        cnt = len(seg_src)
        assert cnt <= B * 128
        base = w * B * 128
        src_arr[base:base + cnt] = seg_src
        i = np.arange(cnt)
        S[w, i % 128, (i // 128) * 128 + seg_dst] = 1.0
        ST[w, seg_dst, (i // 128) * 128 + (i % 128)] = 1.0
    return src_arr, S, ST


def _max_blocks(src, dst):
    best = 0
    for m in range(NCORES):
        lo = m * CHUNK
        sel = (dst >= lo) & (dst < lo + CHUNK)
        ed = dst[sel] - lo
        cnt = np.bincount(ed // WIN, minlength=NW)
        best = max(best, int(np.max((cnt + 127) // 128)))
    return best


def _emit_rsqrt(nc, pool, vcol, P, ncols, coeffs, tag, eng=None):
    """rs = rsqrt(vcol + EPS) via quadratic + 1 Newton step."""
    e = eng if eng is not None else nc.vector
    Q2, Q1, Q0 = coeffs
    rs = pool.tile([128, ncols], F32, tag=f"rs{tag}")
    e.tensor_scalar(rs[:P], vcol, Q2, Q1, ALU.mult, ALU.add)
    e.tensor_mul(rs[:P], rs[:P], vcol)
    e.tensor_scalar(rs[:P], rs[:P], Q0, None, ALU.add)
    vep = pool.tile([128, ncols], F32, tag=f"vep{tag}")
    e.tensor_scalar(vep[:P], vcol, EPS, None, ALU.add)
    t_ = pool.tile([128, ncols], F32, tag=f"tn{tag}")
    e.tensor_mul(t_[:P], rs[:P], rs[:P])
    e.tensor_mul(t_[:P], t_[:P], vep[:P])
    e.tensor_scalar(t_[:P], t_[:P], -0.5, 1.5, ALU.mult, ALU.add)
    e.tensor_mul(rs[:P], rs[:P], t_[:P])
    return rs


def _build_program(B, phases=3):
    nc = bacc.Bacc("TRN2", target_bir_lowering=False, debug=False,
                   num_devices=NCORES)
    BL = B * 128               # padded edges per (window, rel)
    BL2 = 2 * BL
    W16 = BL2 // 16            # idx cols per window (both rels)
    L16 = NW * W16
    RC1 = _rsqrt_coeffs(*_BN1_RANGE)
    RC2 = _rsqrt_coeffs(*_BN2_RANGE)

    # ---- DRAM tensors (all per-core node data ROTATED by -lo mod N) ----
    x_bf = nc.dram_tensor("x_bf", [N, T * D], BF16, kind="ExternalInput")
    xc = nc.dram_tensor("xc", [CHUNK, T * D], BF16, kind="ExternalInput")
    bn1_gb = nc.dram_tensor("bn1_gb", [N, 2], F32, kind="ExternalInput")
    bn2_gb = nc.dram_tensor("bn2_gb", [CHUNK, 2], F32, kind="ExternalInput")
    w_in, al_in, ar_in, s_in, st_in = [], [], [], [], []
    for r in (1, 2):
        w_in.append(nc.dram_tensor(f"W{r}", [D, H * DH], F32, kind="ExternalInput"))
        al_in.append(nc.dram_tensor(f"al{r}t", [D, H * DH], F32, kind="ExternalInput"))
        ar_in.append(nc.dram_tensor(f"ar{r}t", [D, H * DH], F32, kind="ExternalInput"))
        s_in.append(nc.dram_tensor(f"S{r}", [NW, 128, BL], FP8, kind="ExternalInput"))
        st_in.append(nc.dram_tensor(f"ST{r}", [NW, 128, BL], FP8, kind="ExternalInput"))
    si_in = nc.dram_tensor("srcidx", [128, L16], I16, kind="ExternalInput")
    ffw1_in = nc.dram_tensor("ffw1", [D, DFF], F32, kind="ExternalInput")
    ffb1_in = nc.dram_tensor("ffb1", [DFF, 1], F32, kind="ExternalInput")
    ffw2_in = nc.dram_tensor("ffw2", [DFF, D], F32, kind="ExternalInput")
    ffb2_in = nc.dram_tensor("ffb2", [D, 1], F32, kind="ExternalInput")
    ident_in = nc.dram_tensor("ident", [128, 128], BF16, kind="ExternalInput")
    out_d = nc.dram_tensor("OUT", [CHUNK, T * D], BF16, kind="ExternalOutput")

    # interleaved: row 2*rot(node) + rel, 1024B each
    zpackB = nc.dram_tensor("zpackB", [2 * N, ZROW], BF16, kind="Internal")

    with tile.TileContext(nc) as tc:
        with (
            tc.tile_pool(name="const", bufs=1) as cpool,
            tc.tile_pool(name="zg", bufs=PREPD) as zgp,
        ):
            # ---- Phase 0 ----
            ident = cpool.tile([128, 128], BF16)
            nc.sync.dma_start(ident[:], ident_in[:])
            # wpair: fp8, cols [r(2), par(2), 68(=64 z + 4 el)] then er cols
            # 272:288 laid out [par(2), r(2), H] (block-diagonal over par).
            wpair = cpool.tile([128, 288], BF16)
            nc.vector.memset(wpair[:], 0.0)
            for r in range(2):
                wf = cpool.tile([D, H * DH], F32, tag="wf")
                nc.sync.dma_start(wf[:], w_in[r][:])
                for par in range(2):
                    dst = wpair[par * D:(par + 1) * D, :272].rearrange(
                        "p (r q c) -> p r q c", r=2, q=2)
                    nc.vector.tensor_copy(dst[:, r, par, 0:64], wf[:])
                for which, t_in in (("al", al_in[r]), ("ar", ar_in[r])):
                    alt = cpool.tile([D, H * DH], F32, tag="alt")
                    nc.sync.dma_start(alt[:], t_in[:])
                    prod = cpool.tile([D, H * DH], F32, tag="prod")
                    nc.vector.tensor_mul(prod[:], wf[:], alt[:])
                    red = cpool.tile([D, H], F32, tag="red")
                    nc.vector.tensor_reduce(
                        red[:].unsqueeze(2),
                        prod[:].rearrange("p (h k) -> p h k", k=DH),
                        mybir.AxisListType.X, ALU.add)
                    for par in range(2):
                        if which == "al":
                            dst = wpair[par * D:(par + 1) * D, :272].rearrange(
                                "p (r q c) -> p r q c", r=2, q=2)
                            nc.vector.tensor_copy(dst[:, r, par, 64:68], red[:])
                        else:
                            dst = wpair[par * D:(par + 1) * D, 272:288] \
                                .rearrange("p (q r h) -> p q r h", q=2, r=2)
                            nc.vector.tensor_copy(dst[:, par, r, :], red[:])
            ffw1 = cpool.tile([128, DFF], BF16)
            t1 = cpool.tile([D, DFF], F32, tag="t1")
            nc.sync.dma_start(t1[:], ffw1_in[:])
            nc.vector.tensor_copy(ffw1[0:D, :], t1[:])
            nc.sync.dma_start(ffw1[64:128, :], ffw1[0:64, :])
            ffw2 = cpool.tile([DFF, D], BF16)
            t2 = cpool.tile([DFF, D], F32, tag="t2")
            nc.sync.dma_start(t2[:], ffw2_in[:])
            nc.vector.tensor_copy(ffw2[:], t2[:])
            ffb1 = cpool.tile([DFF, 1], F32)
            nc.sync.dma_start(ffb1[:], ffb1_in[:])
            ffb2r = cpool.tile([128, 1], F32)
            nc.sync.dma_start(ffb2r[0:64, :], ffb2_in[:])
            nc.sync.dma_start(ffb2r[64:128, :], ffb2_in[:])
            # per-window er stash: [128, NW, r, T, H] fp8 (tiny)
            erw_all = cpool.tile([128, NW, 2, T, H], FP8)
            nc.vector.memset(erw_all[:], 0.0)

            zgs = {}

            # ---- Phase 1: BN1 + projections for all N nodes (rotated) ----
            with (
                tc.tile_pool(name="p1x", bufs=2) as p1x,
                tc.tile_pool(name="p1z", bufs=3) as p1z,
                tc.tile_pool(name="p1h", bufs=3) as p1h,
                tc.tile_pool(name="p1s", bufs=3) as p1s,
                tc.tile_pool(name="p1tp", bufs=2, space="PSUM") as p1tp,
                tc.tile_pool(name="p1zp", bufs=1, space="PSUM") as p1zp,
            ):
                def super_dma(sb):
                    # batched x/gb DMA for one super-block
                    nsb = min(SUP, NBLK - sb)
                    n0 = sb * 128
                    nn = min(SUP * 128, N - n0)
                    full = (nn == nsb * 128)
                    xt4 = p1x.tile([128, SUP, T * D], BF16, tag="xt4")
                    gbt = p1s.tile([128, SUP, 2], F32, tag="gbt", bufs=2)
                    if full:
                        nc.sync.dma_start(
                            xt4[:, 0:nsb, :],
                            x_bf[n0:n0 + nn].rearrange(
                                "(j p) c -> p j c", p=128))
                        nc.sync.dma_start(
                            gbt[:, 0:nsb, :],
                            bn1_gb[n0:n0 + nn].rearrange(
                                "(j p) c -> p j c", p=128))
                    else:
                        for j in range(nsb):
                            nb = min(128, N - (sb + j) * 128)
                            nc.sync.dma_start(
                                xt4[:nb, j, :],
                                x_bf[(sb + j) * 128:(sb + j) * 128 + nb])
                            nc.sync.dma_start(
                                gbt[:nb, j, :],
                                bn1_gb[(sb + j) * 128:(sb + j) * 128 + nb])
                    mvt = p1s.tile([128, SUP, 2], F32, tag="mvt", bufs=2)
                    return dict(sb=sb, nsb=nsb, xt4=xt4, gbt=gbt, mvt=mvt)

                def super_stats(st, j):
                    # bn stats for block j of a super (interleaved on vector)
                    sb, xt4, mvt = st["sb"], st["xt4"], st["mvt"]
                    nb = min(128, N - (sb + j) * 128)
                    st6 = p1s.tile([128, 2, 6], F32, tag="st6")
                    nc.vector.bn_stats(st6[:nb, 0, :], xt4[:nb, j, 0:384])
                    nc.vector.bn_stats(st6[:nb, 1, :], xt4[:nb, j, 384:768])
                    nc.vector.bn_aggr(mvt[:nb, j, :], st6[:nb])

                def super_finish(st):
                    # rsqrt + affine coefficients (gpsimd)
                    nsb, gbt, mvt = st["nsb"], st["gbt"], st["mvt"]
                    rs = _emit_rsqrt(nc, p1s, mvt[:, 0:nsb, 1], 128, nsb,
                                     RC1, "p1", eng=nc.gpsimd)
                    ab = p1s.tile([128, SUP, 2], F32, tag="ab", bufs=2)
                    nc.gpsimd.tensor_mul(ab[:, 0:nsb, 0], gbt[:, 0:nsb, 0],
                                         rs[:, 0:nsb])
                    nc.gpsimd.tensor_mul(ab[:, 0:nsb, 1], ab[:, 0:nsb, 0],
                                         mvt[:, 0:nsb, 0])
                    nc.gpsimd.tensor_sub(ab[:, 0:nsb, 1], gbt[:, 0:nsb, 1],
                                         ab[:, 0:nsb, 1])
                    st["ab"] = ab

                cur = super_dma(0)
                for j in range(cur["nsb"]):
                    super_stats(cur, j)
                super_finish(cur)
                for sb in range(0, NBLK, SUP):
                    nxt = super_dma(sb + SUP) if sb + SUP < NBLK else None
                    xt4, ab, nsb = cur["xt4"], cur["ab"], cur["nsb"]
                    for j in range(nsb):
                        if nxt is not None and j < nxt["nsb"]:
                            super_stats(nxt, j)
                        blk = sb + j
                        own = blk < NW   # own dst window (er stash)
                        ncol = 288 if own else 272
                        nb = min(128, N - blk * 128)
                        h = p1h.tile([128, T * D], BF16, tag="h")
                        nc.gpsimd.tensor_scalar(
                            h[:nb], xt4[:nb, j, :], ab[:nb, j, 0:1],
                            ab[:nb, j, 1:2], ALU.mult, ALU.add)
                        tp = p1tp.tile([128, NPAIR, 128], BF16, tag="tp")
                        for p in range(NPAIR):
                            nc.tensor.transpose(
                                tp[:, p, 0:nb], h[:nb, p * 128:(p + 1) * 128],
                                ident[:nb, :nb])
                        ht = p1h.tile([128, NPAIR, 128], BF16, tag="ht")
                        nc.scalar.activation(ht[:, :, 0:nb],
                                             tp[:, :, 0:nb], AF.Copy)
                        zel = p1z.tile([128, 2, T * 68], BF16, tag="zel")
                        HP = NPAIR // 2
                        for half in range(2):
                            q0 = half * HP
                            # one PSUM bank (512 f32) per pair; 3 banks/half
                            zp = p1zp.tile([128, HP, 512], F32,
                                           tag=f"zp{half}", bufs=1)
                            for p in range(HP):
                                nc.tensor.matmul(
                                    zp[0:nb, p, 0:ncol],
                                    ht[:, q0 + p, 0:nb],
                                    wpair[:, 0:ncol],
                                    start=True, stop=True)
                            zp_v = zp[:nb, :, 0:272].rearrange(
                                "p q (r par c) -> p q r par c", r=2, c=68)
                            for r in range(2):
                                dst_ap = zel[:nb, r,
                                             q0 * 136:(q0 + HP) * 136] \
                                    .rearrange("p (q par c) -> p q par c",
                                               q=HP, c=68)
                                if r == 0:
                                    nc.scalar.activation(
                                        dst_ap, zp_v[:, :, r], AF.Copy)
                                else:
                                    nc.vector.tensor_copy(
                                        dst_ap, zp_v[:, :, r])
                            if own:
                                nw = _win_nodes(blk)
                                nc.vector.tensor_copy(
                                    erw_all[:nw, blk, :,
                                            q0 * 2:(q0 + HP) * 2, :]
                                    .rearrange("p r (q par) h -> p q par r h",
                                               par=2),
                                    zp[:nw, :, 272:288].rearrange(
                                        "p q (par r h) -> p q par r h",
                                        par=2, r=2))
                        nc.sync.dma_start(
                            zpackB[2 * blk * 128:2 * blk * 128 + 2 * nb,
                                   0:T * 68],
                            zel[:nb].rearrange("p r c -> p (r c)"))
                    if nxt is not None:
                        for j in range(nsb, nxt["nsb"]):
                            super_stats(nxt, j)
                        super_finish(nxt)
                        cur = nxt

            # ---- Phase 2: fused gather/attention/segment-sum/BN2/FFN ----
            # Software-pipelined: the attention front-end (ebc, lk, leaky,
            # exp broadcast, denominator exp) of window w+1 is emitted in
            # iteration w, overlapping the back-end (mz, segment-sum,
            # epilogue, BN2, FFN) of window w.
            with (
                tc.tile_pool(name="x2p", bufs=2) as x2p,
                tc.tile_pool(name="abp", bufs=2) as abp,
                tc.tile_pool(name="sp", bufs=2) as spp,
                tc.tile_pool(name="msg", bufs=2) as msgp,
                tc.tile_pool(name="p2s", bufs=3) as p2s,
                tc.tile_pool(name="p2t", bufs=2) as p2t,
                tc.tile_pool(name="pp", bufs=1, space="PSUM") as pp,
            ):
                pre_f, pre_b, msgbs = {}, {}, {}

                def prefetch_stb(w):
                    ss = []
                    for r in range(2):
                        stb = spp.tile([128, BL], FP8, tag=f"stb{r}", bufs=2)
                        nc.sync.dma_start(stb[:], st_in[r][w])
                        ss.append(stb)
                    pre_f[w] = ss

                sis = {}

                def prefetch_si(w):
                    si = spp.tile([128, W16], I16, tag="si", bufs=4)
                    nc.sync.dma_start(si[:], si_in[:, w * W16:(w + 1) * W16])
                    sis[w] = si

                def prefetch_back(w):
                    nw = _win_nodes(w)
                    xcw = p2t.tile([128, T * D], BF16, tag="xcw")
                    nc.sync.dma_start(xcw[:nw], xc[w * WIN:w * WIN + nw])
                    gb2 = p2s.tile([128, 2], F32, tag="gb2", bufs=2)
                    nc.sync.dma_start(gb2[:nw], bn2_gb[w * WIN:w * WIN + nw])
                    ss = []
                    for r in range(2):
                        ssb = spp.tile([128, BL], FP8, tag=f"ssb{r}")
                        nc.sync.dma_start(ssb[:], s_in[r][w])
                        ss.append(ssb)
                    pre_b[w] = (xcw, gb2, ss)

                def issue_gather(w):
                    zg = zgp.tile([128, 2 * B, ZROW], BF16, tag="zg")
                    si = sis.pop(w)
                    nc.gpsimd.dma_gather(
                        zg[:], zpackB[:], si[:],
                        BL2, BL2, ZROW, single_packet=False)
                    zgs[w] = zg

                lks = {}

                def front_pre(w):
                    # ebc + lk + leaky for window w (emitted one early)
                    zg = zgs[w]
                    stbs = pre_f.pop(w)
                    for r in range(2):
                        stb = stbs[r]
                        lk = p2s.tile([128, B, T * H], BF16, tag="lk",
                                      bufs=2)
                        ebc = pp.tile([128, B, 64], F32, tag="ebc")
                        for b in range(B):
                            nc.tensor.matmul(
                                ebc[:, b, 0:T * H],
                                stb[:, b * 128:(b + 1) * 128],
                                erw_all[:, w, r].rearrange(
                                    "p q h -> p (q h)"),
                                start=True, stop=True)
                        el_ap = zg[:, r * B:(r + 1) * B, 0:T * 68] \
                            .rearrange("p b (t c) -> p b t c",
                                       c=68)[:, :, :, 64:68]
                        nc.vector.tensor_add(
                            lk[:].rearrange("p b (t h) -> p b t h", h=H),
                            el_ap,
                            ebc[:, :, 0:T * H].rearrange(
                                "p b (t h) -> p b t h", h=H))
                        nc.vector.scalar_tensor_tensor(
                            lk[:], lk[:], NEG_SLOPE, lk[:], ALU.mult,
                            ALU.max)
                        lks[(w, r)] = lk

                def front_exp(w):
                    for r in range(2):
                        lk = lks.pop((w, r))
                        msgb = msgp.tile([128, B, 816], BF16,
                                         tag=f"msg{r}", bufs=1)
                        nc.scalar.activation(
                            msgb[:, :, 0:768].rearrange(
                                "p b (t h k) -> p b t h k", h=H, k=DH),
                            lk[:].rearrange("p b (t h) -> p b t h", h=H)
                            .unsqueeze(4).broadcast_to((128, B, T, H, DH)),
                            AF.Exp)
                        nc.scalar.activation(msgb[:, :, 768:816], lk[:],
                                             AF.Exp)
                        msgbs[(w, r)] = msgb

                prefetch_stb(0)
                prefetch_back(0)
                for w in range(min(PREPD, NW)):
                    prefetch_si(w)
                    issue_gather(w)
                for w in range(NW):
                    nw = _win_nodes(w)
                    if w + PREPD < NW:
                        prefetch_si(w + PREPD)
                        issue_gather(w + PREPD)
                    if w + 1 < NW:
                        prefetch_stb(w + 1)
                        prefetch_back(w + 1)
                    xcw, gb2, ssbs = pre_b.pop(w)
                    front_pre(w)
                    front_exp(w)
                    zg = zgs.pop(w)
                    msgbw = []
                    for r in range(2):
                        zap = zg[:, r * B:(r + 1) * B, 0:T * 68].rearrange(
                            "p b (t c) -> p b t c", c=68)[:, :, :, 0:64] \
                            .rearrange("p b t (h k) -> p b t h k", k=DH)
                        msgb = msgbs.pop((w, r))
                        mz = msgb[:, :, 0:768].rearrange(
                            "p b (t h k) -> p b t h k", h=H, k=DH)
                        nc.vector.tensor_mul(mz, mz, zap)
                        msgbw.append(msgb)
                    msum = []
                    for r in range(2):
                        msgb = msgbw[r]
                        ssb = ssbs[r]
                        ms = pp.tile([128, 816], F32, tag="big", bufs=2)
                        for b in range(B):
                            lhsT = ssb[:, b * 128:(b + 1) * 128]
                            nc.tensor.matmul(ms[:, 0:512], lhsT,
                                             msgb[:, b, 0:512],
                                             start=(b == 0),
                                             stop=(b == B - 1))
                            nc.tensor.matmul(ms[:, 512:816], lhsT,
                                             msgb[:, b, 512:816],
                                             start=(b == 0),
                                             stop=(b == B - 1))
                        msum.append(ms)
                    # epilogue: m = msgsum/denom; x2 = bf16(x + m1 + m2)
                    x2w = x2p.tile([128, T * D], BF16, tag="x2")
                    mtmp = p2t.tile([128, T * D], BF16, tag="mtmp")
                    for r in range(2):
                        rec = p2s.tile([128, T * H], F32, tag="rec")
                        nc.vector.tensor_scalar_max(
                            rec[:nw], msum[r][:nw, 768:816], 1e-16)
                        nc.vector.reciprocal(rec[:nw], rec[:nw])
                        rb = rec[:nw].rearrange(
                            "p (t h) -> p t h", h=H).unsqueeze(3) \
                            .broadcast_to((nw, T, H, DH))
                        dst = (mtmp if r == 0 else x2w)
                        nc.vector.tensor_mul(
                            dst[:nw].rearrange(
                                "p (t h k) -> p t h k", h=H, k=DH),
                            msum[r][:nw, 0:768].rearrange(
                                "p (t h k) -> p t h k", h=H, k=DH), rb)
                    nc.vector.tensor_add(mtmp[:nw], mtmp[:nw], xcw[:nw])
                    nc.vector.tensor_add(x2w[:nw], x2w[:nw], mtmp[:nw])
                    # BN2 stats + a2/b2
                    st6b = p2s.tile([128, 2, 6], F32, tag="st6b")
                    nc.vector.bn_stats(st6b[:nw, 0, :], x2w[:nw, 0:384])
                    nc.vector.bn_stats(st6b[:nw, 1, :], x2w[:nw, 384:768])
                    mvb = p2s.tile([128, 2], F32, tag="mvb")
                    nc.vector.bn_aggr(mvb[:nw], st6b[:nw])
                    rs2 = _emit_rsqrt(nc, p2s, mvb[:nw, 1:2], nw, 1, RC2,
                                      "b2")
                    ab2 = abp.tile([128, 2], F32, tag="ab2")
                    nc.vector.tensor_mul(ab2[:nw, 0:1], gb2[:nw, 0:1],
                                         rs2[:nw])
                    nc.vector.tensor_mul(ab2[:nw, 1:2], ab2[:nw, 0:1],
                                         mvb[:nw, 0:1])
                    nc.vector.tensor_sub(ab2[:nw, 1:2], gb2[:nw, 1:2],
                                         ab2[:nw, 1:2])
                    if phases < 3:
                        xo = p2t.tile([128, T * D], BF16, tag="mtmp")
                        nc.vector.tensor_copy(xo[:nw], x2w[:nw])
                        nc.sync.dma_start(out_d[w * WIN:w * WIN + nw],
                                          xo[:nw])
                        continue
                    # ---- FFN: BN2 apply + 2 layers + residual ----
                    h2 = p2t.tile([128, T * D], BF16, tag="hw")
                    nc.scalar.activation(h2[:nw], x2w[:nw], AF.Identity,
                                         bias=ab2[:nw, 1:2],
                                         scale=ab2[:nw, 0:1])
                    h2t = p2t.tile([64, T, 128], BF16, tag="h2t")
                    for half in range(2):
                        tp = pp.tile([64, NPAIR, 128], BF16, tag="tp3")
                        for j in range(NPAIR):
                            t = half * NPAIR + j
                            nc.tensor.transpose(
                                tp[:, j, 0:nw], h2[:nw, t * 64:(t + 1) * 64],
                                ident[:nw, :nw])
                        nc.scalar.activation(
                            h2t[:, half * NPAIR:(half + 1) * NPAIR, 0:nw],
                            tp[:, :, 0:nw], AF.Copy)
                    if nw < 128:
                        nc.vector.memset(h2t[:, :, nw:128], 0.0)
                    dd = pp.tile([128, T, 64], BF16, tag="tpS")
                    fft = p2t.tile([64, T, 128], BF16, tag="fft")
                    for k in range(3):
                        big = pp.tile([128, 816], F32, tag="big", bufs=2)
                        f1 = big[:, 0:512]
                        rhs = h2t[:, 4 * k:4 * k + 4, :]
                        nc.tensor.matmul(f1, ffw1[0:64, :],
                                         rhs.rearrange("p a b -> p (a b)"),
                                         start=True, stop=True)
                        g1 = p2t.tile([128, 512], BF16, tag="g1")
                        nc.scalar.activation(g1[:], f1, AF.Gelu,
                                             bias=ffb1[:])
                        for half in range(2):
                            f2 = big[0:64, 512:768]
                            nc.tensor.matmul(
                                f2, ffw2[:],
                                g1[:, half * 256:(half + 1) * 256],
                                start=True, stop=True)
                            nc.vector.tensor_scalar(
                                fft[:, 4 * k + 2 * half:
                                    4 * k + 2 * half + 2, :]
                                .rearrange("p a b -> p (a b)"),
                                f2, ffb2r[0:64, :], None, ALU.add)
                    for t in range(T):
                        nc.tensor.transpose(
                            dd[0:nw, t, :], fft[:, t, 0:nw],
                            ident[0:64, 0:64])
                    ot = p2t.tile([128, T * D], BF16, tag="mtmp")
                    nc.vector.tensor_add(
                        ot[:nw], dd[:nw].rearrange("p a b -> p (a b)"),
                        x2w[:nw])
                    nc.sync.dma_start(out_d[w * WIN:w * WIN + nw], ot[:nw])

    nc.compile()
    return nc


_CACHE = {}
_PHASES = 3
_TRACE = False
_TRACE_DIR = None
_LAST_EXEC_NS = None


def _host_prep(inputs):
    x = np.asarray(inputs["x"], np.float32)
    xf = np.ascontiguousarray(x.reshape(N, T * D))
    xbf_full = xf.astype(BF16NP)
    B = 0
    for r in (1, 2):
        B = max(B, _max_blocks(np.asarray(inputs[f"src{r}"]),
                               np.asarray(inputs[f"dst{r}"])))

    bn1_gb_full = np.ascontiguousarray(
        np.stack([np.asarray(inputs["bn1_g"], np.float32),
                  np.asarray(inputs["bn1_b"], np.float32)], axis=1))
    bn2_gb_full = np.ascontiguousarray(
        np.stack([np.asarray(inputs["bn2_g"], np.float32),
                  np.asarray(inputs["bn2_b"], np.float32)], axis=1))
    common = {
        "ffw1": np.ascontiguousarray(np.asarray(inputs["ff_w1"], np.float32)),
        "ffb1": np.ascontiguousarray(
            np.asarray(inputs["ff_b1"], np.float32).reshape(DFF, 1)),
        "ffw2": np.ascontiguousarray(np.asarray(inputs["ff_w2"], np.float32)),
        "ffb2": np.ascontiguousarray(
            np.asarray(inputs["ff_b2"], np.float32).reshape(D, 1)),
        "ident": np.eye(128, dtype=BF16NP),
    }
    for r in (1, 2):
        W = np.asarray(inputs[f"W{r}"], np.float32).reshape(D, H * DH)
        al = np.asarray(inputs[f"al{r}"], np.float32).reshape(-1)
        ar = np.asarray(inputs[f"ar{r}"], np.float32).reshape(-1)
        common[f"W{r}"] = np.ascontiguousarray(W)
        common[f"al{r}t"] = np.ascontiguousarray(np.tile(al[None, :], (D, 1)))
        common[f"ar{r}t"] = np.ascontiguousarray(np.tile(ar[None, :], (D, 1)))

    BL = B * 128
    in_maps = []
    for m in range(NCORES):
        lo = m * CHUNK
        im = dict(common)
        # rotated per-core node ordering: row i = node (lo + i) mod N
        im["x_bf"] = np.ascontiguousarray(np.roll(xbf_full, -lo, axis=0))
        im["bn1_gb"] = np.ascontiguousarray(np.roll(bn1_gb_full, -lo, axis=0))
        im["xc"] = np.ascontiguousarray(xbf_full[lo:lo + CHUNK])
        im["bn2_gb"] = np.ascontiguousarray(bn2_gb_full[lo:lo + CHUNK])
        srcs = []
        for r in (1, 2):
            src_flat, S, ST = _prep_core_rel(
                np.asarray(inputs[f"src{r}"]), np.asarray(inputs[f"dst{r}"]),
                lo, B)
            im[f"S{r}"] = S
            im[f"ST{r}"] = ST
            srcs.append(2 * src_flat + (r - 1))  # interleaved zpackB rows
        idx = np.empty((128, NW * (2 * BL) // 16), np.int16)
        for w in range(NW):
            for r in range(2):
                seg = srcs[r][w * BL:(w + 1) * BL].astype(np.int16)
                col0 = (w * 2 + r) * (BL // 16)
                idx[:, col0:col0 + BL // 16] = np.tile(
                    seg.reshape(-1, 16).T, (8, 1))
        im["srcidx"] = np.ascontiguousarray(idx)
        in_maps.append(im)
    return B, in_maps


def kernel(**inputs):
    B, in_maps = _host_prep(inputs)
    key = (B, _PHASES)
    if key not in _CACHE:
        _CACHE[key] = _build_program(B, _PHASES)
    nc = _CACHE[key]
    global _LAST_EXEC_NS
    tmpdir = None
    if _TRACE and _TRACE_DIR:
        import os, shutil
        shutil.rmtree(_TRACE_DIR, ignore_errors=True)
        os.makedirs(_TRACE_DIR, exist_ok=True)
        tmpdir = _TRACE_DIR
    res = run_bass_kernel_spmd(nc, in_maps, core_ids=list(range(NCORES)),
                               trace=_TRACE, tmpdir=tmpdir)
    _LAST_EXEC_NS = res.exec_time_ns
    out = np.concatenate([res.results[m]["OUT"] for m in range(NCORES)],
                         axis=0)
    return out.reshape(N, T, D).astype(np.float32)


# revision 39
# speedup vs baseline: 1.1622x; 1.1622x over previous
"""Trainium2 Bass kernel for nn_EncoderLayer (GNN message passing, 2-relation GAT).

Sharding: nodes (and incoming-edge lists, partitioned by dst) sharded across 8
cores; small GAT/FFN weights replicated; gathered src features fetched from a
replicated projection table via indexed DMA (dma_gather).

v4 layout (per core, node ids ROTATED so own chunk = rows [0, CHUNK)):
  Phase 0: fold weights (fp8 wpair with er columns appended); stage gather
           indices.
  Phase 1: BN1 (bf16 x input, vector rsqrt poly+Newton) + z/el projection for
           ALL N nodes via fp8 matmuls; packed rows
           zpackB[2*rot(node) + rel] = [768 z fp8 | 96B el bf16 | pad] (1024B).
           Blocks 0..9 are this core's own dst windows: er columns (cols
           272:288 of the same matmul) are stashed on-chip in erw_all.
  Phase 2 (per dst-window): gather (plain, 1024B fp8 rows, PREPD-deep ring);
           er broadcast edge-wise via ST fp8 matmul; lk = el(bf16 view) + ebc;
           exp(lk) written fp8 IN-PLACE into the gathered rows' el slot;
           mz = ex (x) z in place (fp8); segment-sum via fp8 one-hot S matmuls
           (816 contiguous fp8 cols = 768 msg + 48 denom); m = msgsum/denom;
           x2 = x + m1 + m2; BN2; FFN interleaved in the same window.
"""

import sys

sys.path.insert(0, "/opt/trn_rl_repo")

import numpy as np
import ml_dtypes

import concourse.bass as bass
import concourse.bacc as bacc
import concourse.tile as tile
import concourse.mybir as mybir
from concourse.bass_utils import run_bass_kernel_spmd

F32 = mybir.dt.float32
BF16 = mybir.dt.bfloat16
FP8 = mybir.dt.float8e4
I16 = mybir.dt.int16
AF = mybir.ActivationFunctionType
ALU = mybir.AluOpType
BF16NP = ml_dtypes.bfloat16
F8NP = ml_dtypes.float8_e4m3

N, T, D, H, DH, DFF = 10000, 12, 64, 4, 16, 128
NCORES = 8
CHUNK = N // NCORES          # 1250
WIN = 128                    # dst-window size (nodes)
NW = (CHUNK + WIN - 1) // WIN  # 10 windows; last has 98 nodes
EPS = 1e-5
NEG_SLOPE = 0.2
ZROW = 896                   # zpack row (bf16 elems): 12*68 data + 80 pad
NBLK = (N + 127) // 128      # 79 phase-1 blocks (last = 16 nodes)
NPAIR = T // 2               # 6 paired (2-timestep) transposes per block
SUP = 8                      # phase-1 super-block (batched DMA + rsqrt math)
PREPD = 3                    # gather ring depth (zg bufs)

# rsqrt(v + EPS) = quadratic fit + one Newton step (vector engine only).
_BN1_RANGE = (0.55, 1.6)
_BN2_RANGE = (0.55, 3.2)


def _rsqrt_coeffs(lo, hi):
    v = np.linspace(lo, hi, 4001)
    c = np.polyfit(v, 1.0 / np.sqrt(v + EPS), 2)
    return [float(x) for x in c]


def _win_nodes(w):
    return min(WIN, CHUNK - w * WIN)


def _prep_core_rel(src, dst, lo, B):
    """Edge lists for one (core, relation): sorted by dst, windowed, padded
    to B blocks of 128 edges per window. Node ids are ROTATED by -lo mod N.
    Returns (src_flat[NW*B*128], S fp8, ST fp8)."""
    hi = lo + CHUNK
    sel = (dst >= lo) & (dst < hi)
    es = ((src[sel].astype(np.int64) - lo) % N)    # rotated src ids
    ed = (dst[sel] - lo).astype(np.int64)
    order = np.argsort(ed, kind="stable")
    es, ed = es[order], ed[order]
    L = NW * B * 128
    src_arr = np.zeros(L, np.int64)
    S = np.zeros((NW, 128, B * 128), F8NP)
    ST = np.zeros((NW, 128, B * 128), F8NP)
    wstart = np.searchsorted(ed, np.arange(NW) * WIN)
    wend = np.searchsorted(ed, np.arange(1, NW + 1) * WIN)
    for w in range(NW):
        seg_src = es[wstart[w]:wend[w]]
        seg_dst = ed[wstart[w]:wend[w]] - w * WIN
        cnt = len(seg_src)
        assert cnt <= B * 128
        base = w * B * 128
        src_arr[base:base + cnt] = seg_src
        i = np.arange(cnt)
        S[w, i % 128, (i // 128) * 128 + seg_dst] = 1.0
        ST[w, seg_dst, (i // 128) * 128 + (i % 128)] = 1.0
    return src_arr, S, ST


def _max_blocks(src, dst):
    best = 0
    for m in range(NCORES):
        lo = m * CHUNK
        sel = (dst >= lo) & (dst < lo + CHUNK)
        ed = dst[sel] - lo
        cnt = np.bincount(ed // WIN, minlength=NW)
        best = max(best, int(np.max((cnt + 127) // 128)))
    return best


def _emit_rsqrt(nc, pool, vcol, P, ncols, coeffs, tag, eng=None):
    """rs = rsqrt(vcol + EPS) via quadratic + 1 Newton step."""
    e = eng if eng is not None else nc.vector
    Q2, Q1, Q0 = coeffs
    rs = pool.tile([128, ncols], F32, tag=f"rs{tag}")
    e.tensor_scalar(rs[:P], vcol, Q2, Q1, ALU.mult, ALU.add)
    e.tensor_mul(rs[:P], rs[:P], vcol)
    e.tensor_scalar(rs[:P], rs[:P], Q0, None, ALU.add)
    vep = pool.tile([128, ncols], F32, tag=f"vep{tag}")
    e.tensor_scalar(vep[:P], vcol, EPS, None, ALU.add)
    t_ = pool.tile([128, ncols], F32, tag=f"tn{tag}")
    e.tensor_mul(t_[:P], rs[:P], rs[:P])
    e.tensor_mul(t_[:P], t_[:P], vep[:P])
    e.tensor_scalar(t_[:P], t_[:P], -0.5, 1.5, ALU.mult, ALU.add)
    e.tensor_mul(rs[:P], rs[:P], t_[:P])
    return rs


def _build_program(B, phases=3):
    nc = bacc.Bacc("TRN2", target_bir_lowering=False, debug=False,
                   num_devices=NCORES)
    BL = B * 128               # padded edges per (window, rel)
    BL2 = 2 * BL
    W16 = BL2 // 16            # idx cols per window (both rels)
    L16 = NW * W16
    RC1 = _rsqrt_coeffs(*_BN1_RANGE)
    RC2 = _rsqrt_coeffs(*_BN2_RANGE)

    # ---- DRAM tensors (all per-core node data ROTATED by -lo mod N) ----
    x_bf = nc.dram_tensor("x_bf", [N, T * D], BF16, kind="ExternalInput")
    xc = nc.dram_tensor("xc", [CHUNK, T * D], BF16, kind="ExternalInput")
    bn1_gb = nc.dram_tensor("bn1_gb", [N, 2], F32, kind="ExternalInput")
    bn2_gb = nc.dram_tensor("bn2_gb", [CHUNK, 2], F32, kind="ExternalInput")
    w_in, al_in, ar_in, s_in, st_in = [], [], [], [], []
    for r in (1, 2):
        w_in.append(nc.dram_tensor(f"W{r}", [D, H * DH], F32, kind="ExternalInput"))
        al_in.append(nc.dram_tensor(f"al{r}t", [D, H * DH], F32, kind="ExternalInput"))
        ar_in.append(nc.dram_tensor(f"ar{r}t", [D, H * DH], F32, kind="ExternalInput"))
        s_in.append(nc.dram_tensor(f"S{r}", [NW, 128, BL], FP8, kind="ExternalInput"))
        st_in.append(nc.dram_tensor(f"ST{r}", [NW, 128, BL], FP8, kind="ExternalInput"))
    si_in = nc.dram_tensor("srcidx", [128, L16], I16, kind="ExternalInput")
    ffw1_in = nc.dram_tensor("ffw1", [D, DFF], F32, kind="ExternalInput")
    ffb1_in = nc.dram_tensor("ffb1", [DFF, 1], F32, kind="ExternalInput")
    ffw2_in = nc.dram_tensor("ffw2", [DFF, D], F32, kind="ExternalInput")
    ffb2_in = nc.dram_tensor("ffb2", [D, 1], F32, kind="ExternalInput")
    ident_in = nc.dram_tensor("ident", [128, 128], BF16, kind="ExternalInput")
    out_d = nc.dram_tensor("OUT", [CHUNK, T * D], BF16, kind="ExternalOutput")

    # interleaved: row 2*rot(node) + rel, 1024B each
    zpackB = nc.dram_tensor("zpackB", [2 * N, ZROW], BF16, kind="Internal")

    with tile.TileContext(nc) as tc:
        with (
            tc.tile_pool(name="const", bufs=1) as cpool,
            tc.tile_pool(name="zg", bufs=PREPD) as zgp,
        ):
            # ---- Phase 0 ----
            ident = cpool.tile([128, 128], BF16)
            nc.sync.dma_start(ident[:], ident_in[:])
            # wpair: fp8, cols [r(2), par(2), 68(=64 z + 4 el)] then er cols
            # 272:288 laid out [par(2), r(2), H] (block-diagonal over par).
            wpair = cpool.tile([128, 288], BF16)
            nc.vector.memset(wpair[:], 0.0)
            for r in range(2):
                wf = cpool.tile([D, H * DH], F32, tag="wf")
                nc.sync.dma_start(wf[:], w_in[r][:])
                for par in range(2):
                    dst = wpair[par * D:(par + 1) * D, :272].rearrange(
                        "p (r q c) -> p r q c", r=2, q=2)
                    nc.vector.tensor_copy(dst[:, r, par, 0:64], wf[:])
                for which, t_in in (("al", al_in[r]), ("ar", ar_in[r])):
                    alt = cpool.tile([D, H * DH], F32, tag="alt")
                    nc.sync.dma_start(alt[:], t_in[:])
                    prod = cpool.tile([D, H * DH], F32, tag="prod")
                    nc.vector.tensor_mul(prod[:], wf[:], alt[:])
                    red = cpool.tile([D, H], F32, tag="red")
                    nc.vector.tensor_reduce(
                        red[:].unsqueeze(2),
                        prod[:].rearrange("p (h k) -> p h k", k=DH),
                        mybir.AxisListType.X, ALU.add)
                    for par in range(2):
                        if which == "al":
                            dst = wpair[par * D:(par + 1) * D, :272].rearrange(
                                "p (r q c) -> p r q c", r=2, q=2)
                            nc.vector.tensor_copy(dst[:, r, par, 64:68], red[:])
                        else:
                            dst = wpair[par * D:(par + 1) * D, 272:288] \
                                .rearrange("p (q r h) -> p q r h", q=2, r=2)
                            nc.vector.tensor_copy(dst[:, par, r, :], red[:])
            ffw1 = cpool.tile([128, DFF], BF16)
            t1 = cpool.tile([D, DFF], F32, tag="t1")
            nc.sync.dma_start(t1[:], ffw1_in[:])
            nc.vector.tensor_copy(ffw1[0:D, :], t1[:])
            nc.sync.dma_start(ffw1[64:128, :], ffw1[0:64, :])
            ffw2 = cpool.tile([DFF, D], BF16)
            t2 = cpool.tile([DFF, D], F32, tag="t2")
            nc.sync.dma_start(t2[:], ffw2_in[:])
            nc.vector.tensor_copy(ffw2[:], t2[:])
            ffb1 = cpool.tile([DFF, 1], F32)
            nc.sync.dma_start(ffb1[:], ffb1_in[:])
            ffb2r = cpool.tile([128, 1], F32)
            nc.sync.dma_start(ffb2r[0:64, :], ffb2_in[:])
            nc.sync.dma_start(ffb2r[64:128, :], ffb2_in[:])
            # per-window er stash: [128, NW, r, T, H] fp8 (tiny)
            erw_all = cpool.tile([128, NW, 2, T, H], FP8)
            nc.vector.memset(erw_all[:], 0.0)

            zgs = {}

            # ---- Phase 1: BN1 + projections for all N nodes (rotated) ----
            with (
                tc.tile_pool(name="p1x", bufs=2) as p1x,
                tc.tile_pool(name="p1z", bufs=3) as p1z,
                tc.tile_pool(name="p1h", bufs=3) as p1h,
                tc.tile_pool(name="p1s", bufs=3) as p1s,
                tc.tile_pool(name="p1tp", bufs=2, space="PSUM") as p1tp,
                tc.tile_pool(name="p1zp", bufs=1, space="PSUM") as p1zp,
            ):
                def stage_a(sb):
                    # batched x/gb DMA + stats + rsqrt poly for one super
                    nsb = min(SUP, NBLK - sb)
                    n0 = sb * 128
                    nn = min(SUP * 128, N - n0)
                    full = (nn == nsb * 128)
                    xt4 = p1x.tile([128, SUP, T * D], BF16, tag="xt4")
                    gbt = p1s.tile([128, SUP, 2], F32, tag="gbt")
                    if full:
                        nc.sync.dma_start(
                            xt4[:, 0:nsb, :],
                            x_bf[n0:n0 + nn].rearrange(
                                "(j p) c -> p j c", p=128))
                        nc.sync.dma_start(
                            gbt[:, 0:nsb, :],
                            bn1_gb[n0:n0 + nn].rearrange(
                                "(j p) c -> p j c", p=128))
                    mvt = p1s.tile([128, SUP, 2], F32, tag="mvt")
                    for j in range(nsb):
                        nb = min(128, N - (sb + j) * 128)
                        if not full:
                            nc.sync.dma_start(
                                xt4[:nb, j, :],
                                x_bf[(sb + j) * 128:(sb + j) * 128 + nb])
                            nc.sync.dma_start(
                                gbt[:nb, j, :],
                                bn1_gb[(sb + j) * 128:(sb + j) * 128 + nb])
                        st6 = p1s.tile([128, 2, 6], F32, tag="st6")
                        nc.vector.bn_stats(st6[:nb, 0, :], xt4[:nb, j, 0:384])
                        nc.vector.bn_stats(st6[:nb, 1, :],
                                           xt4[:nb, j, 384:768])
                        nc.vector.bn_aggr(mvt[:nb, j, :], st6[:nb])
                    rs = _emit_rsqrt(nc, p1s, mvt[:, 0:nsb, 1], 128, nsb,
                                     RC1, "p1", eng=nc.gpsimd)
                    ab = p1s.tile([128, SUP, 2], F32, tag="ab")
                    nc.gpsimd.tensor_mul(ab[:, 0:nsb, 0], gbt[:, 0:nsb, 0],
                                         rs[:, 0:nsb])
                    nc.gpsimd.tensor_mul(ab[:, 0:nsb, 1], ab[:, 0:nsb, 0],
                                         mvt[:, 0:nsb, 0])
                    nc.gpsimd.tensor_sub(ab[:, 0:nsb, 1], gbt[:, 0:nsb, 1],
                                         ab[:, 0:nsb, 1])
                    return xt4, ab, nsb

                for sb in range(0, NBLK, SUP):
                    xt4, ab, nsb = stage_a(sb)
                    for j in range(nsb):
                        blk = sb + j
                        own = blk < NW   # own dst window (er stash)
                        ncol = 288 if own else 272
                        nb = min(128, N - blk * 128)
                        h = p1h.tile([128, T * D], BF16, tag="h")
                        nc.gpsimd.tensor_scalar(
                            h[:nb], xt4[:nb, j, :], ab[:nb, j, 0:1],
                            ab[:nb, j, 1:2], ALU.mult, ALU.add)
                        tp = p1tp.tile([128, NPAIR, 128], BF16, tag="tp")
                        for p in range(NPAIR):
                            nc.tensor.transpose(
                                tp[:, p, 0:nb], h[:nb, p * 128:(p + 1) * 128],
                                ident[:nb, :nb])
                        ht = p1h.tile([128, NPAIR, 128], BF16, tag="ht")
                        nc.scalar.activation(ht[:, :, 0:nb],
                                             tp[:, :, 0:nb], AF.Copy)
                        zel = p1z.tile([128, 2, T * 68], BF16, tag="zel")
                        HP = NPAIR // 2
                        for half in range(2):
                            q0 = half * HP
                            # one PSUM bank (512 f32) per pair; 3 banks/half
                            zp = p1zp.tile([128, HP, 512], F32,
                                           tag=f"zp{half}", bufs=1)
                            for p in range(HP):
                                nc.tensor.matmul(
                                    zp[0:nb, p, 0:ncol],
                                    ht[:, q0 + p, 0:nb],
                                    wpair[:, 0:ncol],
                                    start=True, stop=True)
                            zp_v = zp[:nb, :, 0:272].rearrange(
                                "p q (r par c) -> p q r par c", r=2, c=68)
                            for r in range(2):
                                dst_ap = zel[:nb, r,
                                             q0 * 136:(q0 + HP) * 136] \
                                    .rearrange("p (q par c) -> p q par c",
                                               q=HP, c=68)
                                if r == 0:
                                    nc.scalar.activation(
                                        dst_ap, zp_v[:, :, r], AF.Copy)
                                else:
                                    nc.vector.tensor_copy(
                                        dst_ap, zp_v[:, :, r])
                            if own:
                                nw = _win_nodes(blk)
                                nc.vector.tensor_copy(
                                    erw_all[:nw, blk, :,
                                            q0 * 2:(q0 + HP) * 2, :]
                                    .rearrange("p r (q par) h -> p q par r h",
                                               par=2),
                                    zp[:nw, :, 272:288].rearrange(
                                        "p q (par r h) -> p q par r h",
                                        par=2, r=2))
                        nc.sync.dma_start(
                            zpackB[2 * blk * 128:2 * blk * 128 + 2 * nb,
                                   0:T * 68],
                            zel[:nb].rearrange("p r c -> p (r c)"))

            # ---- Phase 2: fused gather/attention/segment-sum/BN2/FFN ----
            # Software-pipelined: the attention front-end (ebc, lk, leaky,
            # exp broadcast, denominator exp) of window w+1 is emitted in
            # iteration w, overlapping the back-end (mz, segment-sum,
            # epilogue, BN2, FFN) of window w.
            with (
                tc.tile_pool(name="x2p", bufs=2) as x2p,
                tc.tile_pool(name="abp", bufs=2) as abp,
                tc.tile_pool(name="sp", bufs=2) as spp,
                tc.tile_pool(name="msg", bufs=2) as msgp,
                tc.tile_pool(name="p2s", bufs=3) as p2s,
                tc.tile_pool(name="p2t", bufs=2) as p2t,
                tc.tile_pool(name="pp", bufs=1, space="PSUM") as pp,
            ):
                pre_f, pre_b, msgbs = {}, {}, {}

                def prefetch_stb(w):
                    ss = []
                    for r in range(2):
                        stb = spp.tile([128, BL], FP8, tag=f"stb{r}", bufs=2)
                        nc.sync.dma_start(stb[:], st_in[r][w])
                        ss.append(stb)
                    pre_f[w] = ss

                sis = {}

                def prefetch_si(w):
                    si = spp.tile([128, W16], I16, tag="si", bufs=4)
                    nc.sync.dma_start(si[:], si_in[:, w * W16:(w + 1) * W16])
                    sis[w] = si

                def prefetch_back(w):
                    nw = _win_nodes(w)
                    xcw = p2t.tile([128, T * D], BF16, tag="xcw")
                    nc.sync.dma_start(xcw[:nw], xc[w * WIN:w * WIN + nw])
                    gb2 = p2s.tile([128, 2], F32, tag="gb2", bufs=2)
                    nc.sync.dma_start(gb2[:nw], bn2_gb[w * WIN:w * WIN + nw])
                    ss = []
                    for r in range(2):
                        ssb = spp.tile([128, BL], FP8, tag=f"ssb{r}")
                        nc.sync.dma_start(ssb[:], s_in[r][w])
                        ss.append(ssb)
                    pre_b[w] = (xcw, gb2, ss)

                def issue_gather(w):
                    zg = zgp.tile([128, 2 * B, ZROW], BF16, tag="zg")
                    si = sis.pop(w)
                    nc.gpsimd.dma_gather(
                        zg[:], zpackB[:], si[:],
                        BL2, BL2, ZROW, single_packet=False)
                    zgs[w] = zg

                lks = {}

                def front_pre(w):
                    # ebc + lk + leaky for window w (emitted one early)
                    zg = zgs[w]
                    stbs = pre_f.pop(w)
                    for r in range(2):
                        stb = stbs[r]
                        lk = p2s.tile([128, B, T * H], BF16, tag="lk",
                                      bufs=2)
                        ebc = pp.tile([128, B, 64], F32, tag="ebc")
                        for b in range(B):
                            nc.tensor.matmul(
                                ebc[:, b, 0:T * H],
                                stb[:, b * 128:(b + 1) * 128],
                                erw_all[:, w, r].rearrange(
                                    "p q h -> p (q h)"),
                                start=True, stop=True)
                        el_ap = zg[:, r * B:(r + 1) * B, 0:T * 68] \
                            .rearrange("p b (t c) -> p b t c",
                                       c=68)[:, :, :, 64:68]
                        nc.vector.tensor_add(
                            lk[:].rearrange("p b (t h) -> p b t h", h=H),
                            el_ap,
                            ebc[:, :, 0:T * H].rearrange(
                                "p b (t h) -> p b t h", h=H))
                        nc.vector.scalar_tensor_tensor(
                            lk[:], lk[:], NEG_SLOPE, lk[:], ALU.mult,
                            ALU.max)
                        lks[(w, r)] = lk

                def front_exp(w):
                    for r in range(2):
                        lk = lks.pop((w, r))
                        msgb = msgp.tile([128, B, 816], BF16,
                                         tag=f"msg{r}", bufs=1)
                        nc.scalar.activation(
                            msgb[:, :, 0:768].rearrange(
                                "p b (t h k) -> p b t h k", h=H, k=DH),
                            lk[:].rearrange("p b (t h) -> p b t h", h=H)
                            .unsqueeze(4).broadcast_to((128, B, T, H, DH)),
                            AF.Exp)
                        nc.scalar.activation(msgb[:, :, 768:816], lk[:],
                                             AF.Exp)
                        msgbs[(w, r)] = msgb

                prefetch_stb(0)
                prefetch_back(0)
                for w in range(min(PREPD, NW)):
                    prefetch_si(w)
                    issue_gather(w)
                for w in range(NW):
                    nw = _win_nodes(w)
                    if w + PREPD < NW:
                        prefetch_si(w + PREPD)
                        issue_gather(w + PREPD)
                    if w + 1 < NW:
                        prefetch_stb(w + 1)
                        prefetch_back(w + 1)
                    xcw, gb2, ssbs = pre_b.pop(w)
                    front_pre(w)
                    front_exp(w)
                    zg = zgs.pop(w)
                    msgbw = []
                    for r in range(2):
                        zap = zg[:, r * B:(r + 1) * B, 0:T * 68].rearrange(
                            "p b (t c) -> p b t c", c=68)[:, :, :, 0:64] \
                            .rearrange("p b t (h k) -> p b t h k", k=DH)
                        msgb = msgbs.pop((w, r))
                        mz = msgb[:, :, 0:768].rearrange(
                            "p b (t h k) -> p b t h k", h=H, k=DH)
                        nc.vector.tensor_mul(mz, mz, zap)
                        msgbw.append(msgb)
                    msum = []
                    for r in range(2):
                        msgb = msgbw[r]
                        ssb = ssbs[r]
                        ms = pp.tile([128, 816], F32, tag="big", bufs=2)
                        for b in range(B):
                            lhsT = ssb[:, b * 128:(b + 1) * 128]
                            nc.tensor.matmul(ms[:, 0:512], lhsT,
                                             msgb[:, b, 0:512],
                                             start=(b == 0),
                                             stop=(b == B - 1))
                            nc.tensor.matmul(ms[:, 512:816], lhsT,
                                             msgb[:, b, 512:816],
                                             start=(b == 0),
                                             stop=(b == B - 1))
                        msum.append(ms)
                    # epilogue: m = msgsum/denom; x2 = bf16(x + m1 + m2)
                    x2w = x2p.tile([128, T * D], BF16, tag="x2")
                    mtmp = p2t.tile([128, T * D], BF16, tag="mtmp")
                    for r in range(2):
                        rec = p2s.tile([128, T * H], F32, tag="rec")
                        nc.vector.tensor_scalar_max(
                            rec[:nw], msum[r][:nw, 768:816], 1e-16)
                        nc.vector.reciprocal(rec[:nw], rec[:nw])
                        rb = rec[:nw].rearrange(
                            "p (t h) -> p t h", h=H).unsqueeze(3) \
                            .broadcast_to((nw, T, H, DH))
                        dst = (mtmp if r == 0 else x2w)
                        nc.vector.tensor_mul(
                            dst[:nw].rearrange(
                                "p (t h k) -> p t h k", h=H, k=DH),
                            msum[r][:nw, 0:768].rearrange(
                                "p (t h k) -> p t h k", h=H, k=DH), rb)
                    nc.vector.tensor_add(mtmp[:nw], mtmp[:nw], xcw[:nw])
                    nc.vector.tensor_add(x2w[:nw], x2w[:nw], mtmp[:nw])
                    # BN2 stats + a2/b2
                    st6b = p2s.tile([128, 2, 6], F32, tag="st6b")
                    nc.vector.bn_stats(st6b[:nw, 0, :], x2w[:nw, 0:384])
                    nc.vector.bn_stats(st6b[:nw, 1, :], x2w[:nw, 384:768])
                    mvb = p2s.tile([128, 2], F32, tag="mvb")
                    nc.vector.bn_aggr(mvb[:nw], st6b[:nw])
                    rs2 = _emit_rsqrt(nc, p2s, mvb[:nw, 1:2], nw, 1, RC2,
                                      "b2")
                    ab2 = abp.tile([128, 2], F32, tag="ab2")
                    nc.vector.tensor_mul(ab2[:nw, 0:1], gb2[:nw, 0:1],
                                         rs2[:nw])
                    nc.vector.tensor_mul(ab2[:nw, 1:2], ab2[:nw, 0:1],
                                         mvb[:nw, 0:1])
                    nc.vector.tensor_sub(ab2[:nw, 1:2], gb2[:nw, 1:2],
                                         ab2[:nw, 1:2])
                    if phases < 3:
                        xo = p2t.tile([128, T * D], BF16, tag="mtmp")
                        nc.vector.tensor_copy(xo[:nw], x2w[:nw])
                        nc.sync.dma_start(out_d[w * WIN:w * WIN + nw],
                                          xo[:nw])
                        continue
                    # ---- FFN: BN2 apply + 2 layers + residual ----
                    h2 = p2t.tile([128, T * D], BF16, tag="hw")
                    nc.scalar.activation(h2[:nw], x2w[:nw], AF.Identity,
                                         bias=ab2[:nw, 1:2],
                                         scale=ab2[:nw, 0:1])
                    h2t = p2t.tile([64, T, 128], BF16, tag="h2t")
                    for half in range(2):
                        tp = pp.tile([64, NPAIR, 128], BF16, tag="tp3")
                        for j in range(NPAIR):
                            t = half * NPAIR + j
                            nc.tensor.transpose(
                                tp[:, j, 0:nw], h2[:nw, t * 64:(t + 1) * 64],
                                ident[:nw, :nw])
                        nc.scalar.activation(
                            h2t[:, half * NPAIR:(half + 1) * NPAIR, 0:nw],
                            tp[:, :, 0:nw], AF.Copy)
                    if nw < 128:
                        nc.vector.memset(h2t[:, :, nw:128], 0.0)
                    dd = pp.tile([128, T, 64], BF16, tag="tpS")
                    fft = p2t.tile([64, T, 128], BF16, tag="fft")
                    for k in range(3):
                        big = pp.tile([128, 816], F32, tag="big", bufs=2)
                        f1 = big[:, 0:512]
                        rhs = h2t[:, 4 * k:4 * k + 4, :]
                        nc.tensor.matmul(f1, ffw1[0:64, :],
                                         rhs.rearrange("p a b -> p (a b)"),
                                         start=True, stop=True)
                        g1 = p2t.tile([128, 512], BF16, tag="g1")
                        nc.scalar.activation(g1[:], f1, AF.Gelu,
                                             bias=ffb1[:])
                        for half in range(2):
                            f2 = big[0:64, 512:768]
                            nc.tensor.matmul(
                                f2, ffw2[:],
                                g1[:, half * 256:(half + 1) * 256],
                                start=True, stop=True)
                            nc.vector.tensor_scalar(
                                fft[:, 4 * k + 2 * half:
                                    4 * k + 2 * half + 2, :]
                                .rearrange("p a b -> p (a b)"),
                                f2, ffb2r[0:64, :], None, ALU.add)
                    for t in range(T):
                        nc.tensor.transpose(
                            dd[0:nw, t, :], fft[:, t, 0:nw],
                            ident[0:64, 0:64])
                    ot = p2t.tile([128, T * D], BF16, tag="mtmp")
                    nc.vector.tensor_add(
                        ot[:nw], dd[:nw].rearrange("p a b -> p (a b)"),
                        x2w[:nw])
                    nc.sync.dma_start(out_d[w * WIN:w * WIN + nw], ot[:nw])

    nc.compile()
    return nc


_CACHE = {}
_PHASES = 3
_TRACE = False
_TRACE_DIR = None
_LAST_EXEC_NS = None


def _host_prep(inputs):
    x = np.asarray(inputs["x"], np.float32)
    xf = np.ascontiguousarray(x.reshape(N, T * D))
    xbf_full = xf.astype(BF16NP)
    B = 0
    for r in (1, 2):
        B = max(B, _max_blocks(np.asarray(inputs[f"src{r}"]),
                               np.asarray(inputs[f"dst{r}"])))

    bn1_gb_full = np.ascontiguousarray(
        np.stack([np.asarray(inputs["bn1_g"], np.float32),
                  np.asarray(inputs["bn1_b"], np.float32)], axis=1))
    bn2_gb_full = np.ascontiguousarray(
        np.stack([np.asarray(inputs["bn2_g"], np.float32),
                  np.asarray(inputs["bn2_b"], np.float32)], axis=1))
    common = {
        "ffw1": np.ascontiguousarray(np.asarray(inputs["ff_w1"], np.float32)),
        "ffb1": np.ascontiguousarray(
            np.asarray(inputs["ff_b1"], np.float32).reshape(DFF, 1)),
        "ffw2": np.ascontiguousarray(np.asarray(inputs["ff_w2"], np.float32)),
        "ffb2": np.ascontiguousarray(
            np.asarray(inputs["ff_b2"], np.float32).reshape(D, 1)),
        "ident": np.eye(128, dtype=BF16NP),
    }
    for r in (1, 2):
        W = np.asarray(inputs[f"W{r}"], np.float32).reshape(D, H * DH)
        al = np.asarray(inputs[f"al{r}"], np.float32).reshape(-1)
        ar = np.asarray(inputs[f"ar{r}"], np.float32).reshape(-1)
        common[f"W{r}"] = np.ascontiguousarray(W)
        common[f"al{r}t"] = np.ascontiguousarray(np.tile(al[None, :], (D, 1)))
        common[f"ar{r}t"] = np.ascontiguousarray(np.tile(ar[None, :], (D, 1)))

    BL = B * 128
    in_maps = []
    for m in range(NCORES):
        lo = m * CHUNK
        im = dict(common)
        # rotated per-core node ordering: row i = node (lo + i) mod N
        im["x_bf"] = np.ascontiguousarray(np.roll(xbf_full, -lo, axis=0))
        im["bn1_gb"] = np.ascontiguousarray(np.roll(bn1_gb_full, -lo, axis=0))
        im["xc"] = np.ascontiguousarray(xbf_full[lo:lo + CHUNK])
        im["bn2_gb"] = np.ascontiguousarray(bn2_gb_full[lo:lo + CHUNK])
        srcs = []
        for r in (1, 2):
            src_flat, S, ST = _prep_core_rel(
                np.asarray(inputs[f"src{r}"]), np.asarray(inputs[f"dst{r}"]),
                lo, B)
            im[f"S{r}"] = S
            im[f"ST{r}"] = ST
            srcs.append(2 * src_flat + (r - 1))  # interleaved zpackB rows
        idx = np.empty((128, NW * (2 * BL) // 16), np.int16)
        for w in range(NW):
            for r in range(2):
                seg = srcs[r][w * BL:(w + 1) * BL].astype(np.int16)
                col0 = (w * 2 + r) * (BL // 16)
                idx[:, col0:col0 + BL // 16] = np.tile(
                    seg.reshape(-1, 16).T, (8, 1))
        im["srcidx"] = np.ascontiguousarray(idx)
        in_maps.append(im)
    return B, in_maps


def kernel(**inputs):
    B, in_maps = _host_prep(inputs)
    key = (B, _PHASES)
    if key not in _CACHE:
        _CACHE[key] = _build_program(B, _PHASES)
    nc = _CACHE[key]
    global _LAST_EXEC_NS
    tmpdir = None
    if _TRACE and _TRACE_DIR:
        import os, shutil
        shutil.rmtree(_TRACE_DIR, ignore_errors=True)
        os.makedirs(_TRACE_DIR, exist_ok=True)
        tmpdir = _TRACE_DIR
    res = run_bass_kernel_spmd(nc, in_maps, core_ids=list(range(NCORES)),
                               trace=_TRACE, tmpdir=tmpdir)
    _LAST_EXEC_NS = res.exec_time_ns
    out = np.concatenate([res.results[m]["OUT"] for m in range(NCORES)],
                         axis=0)
    return out.reshape(N, T, D).astype(np.float32)


# revision 41
# speedup vs baseline: 1.1623x; 1.0000x over previous
"""Trainium2 Bass kernel for nn_EncoderLayer (GNN message passing, 2-relation GAT).

Sharding: nodes (and incoming-edge lists, partitioned by dst) sharded across 8
cores; small GAT/FFN weights replicated; gathered src features fetched from a
replicated projection table via indexed DMA (dma_gather).

v4 layout (per core, node ids ROTATED so own chunk = rows [0, CHUNK)):
  Phase 0: fold weights (fp8 wpair with er columns appended); stage gather
           indices.
  Phase 1: BN1 (bf16 x input, vector rsqrt poly+Newton) + z/el projection for
           ALL N nodes via fp8 matmuls; packed rows
           zpackB[2*rot(node) + rel] = [768 z fp8 | 96B el bf16 | pad] (1024B).
           Blocks 0..9 are this core's own dst windows: er columns (cols
           272:288 of the same matmul) are stashed on-chip in erw_all.
  Phase 2 (per dst-window): gather (plain, 1024B fp8 rows, PREPD-deep ring);
           er broadcast edge-wise via ST fp8 matmul; lk = el(bf16 view) + ebc;
           exp(lk) written fp8 IN-PLACE into the gathered rows' el slot;
           mz = ex (x) z in place (fp8); segment-sum via fp8 one-hot S matmuls
           (816 contiguous fp8 cols = 768 msg + 48 denom); m = msgsum/denom;
           x2 = x + m1 + m2; BN2; FFN interleaved in the same window.
"""

import sys

sys.path.insert(0, "/opt/trn_rl_repo")

import numpy as np
import ml_dtypes

import concourse.bass as bass
import concourse.bacc as bacc
import concourse.tile as tile
import concourse.mybir as mybir
from concourse.bass_utils import run_bass_kernel_spmd

F32 = mybir.dt.float32
BF16 = mybir.dt.bfloat16
FP8 = mybir.dt.float8e4
I16 = mybir.dt.int16
AF = mybir.ActivationFunctionType
ALU = mybir.AluOpType
BF16NP = ml_dtypes.bfloat16
F8NP = ml_dtypes.float8_e4m3

N, T, D, H, DH, DFF = 10000, 12, 64, 4, 16, 128
NCORES = 8
CHUNK = N // NCORES          # 1250
WIN = 128                    # dst-window size (nodes)
NW = (CHUNK + WIN - 1) // WIN  # 10 windows; last has 98 nodes
EPS = 1e-5
NEG_SLOPE = 0.2
ZROW = 896                   # zpack row (bf16 elems): 12*68 data + 80 pad
NBLK = (N + 127) // 128      # 79 phase-1 blocks (last = 16 nodes)
NPAIR = T // 2               # 6 paired (2-timestep) transposes per block
SUP = 8                      # phase-1 super-block (batched DMA + rsqrt math)
PREPD = 3                    # gather ring depth (zg bufs)

# rsqrt(v + EPS) = quadratic fit + one Newton step (vector engine only).
_BN1_RANGE = (0.55, 1.6)
_BN2_RANGE = (0.55, 3.2)


def _rsqrt_coeffs(lo, hi):
    v = np.linspace(lo, hi, 4001)
    c = np.polyfit(v, 1.0 / np.sqrt(v + EPS), 2)
    return [float(x) for x in c]


def _win_nodes(w):
    return min(WIN, CHUNK - w * WIN)


def _prep_core_rel(src, dst, lo, B):
    """Edge lists for one (core, relation): sorted by dst, windowed, padded
    to B blocks of 128 edges per window. Node ids are ROTATED by -lo mod N.
    Returns (src_flat[NW*B*128], S fp8, ST fp8)."""
    hi = lo + CHUNK
    sel = (dst >= lo) & (dst < hi)
    es = ((src[sel].astype(np.int64) - lo) % N)    # rotated src ids
    ed = (dst[sel] - lo).astype(np.int64)
    order = np.argsort(ed, kind="stable")
    es, ed = es[order], ed[order]
    L = NW * B * 128
    src_arr = np.zeros(L, np.int64)
    S = np.zeros((NW, 128, B * 128), F8NP)
    ST = np.zeros((NW, 128, B * 128), F8NP)
    wstart = np.searchsorted(ed, np.arange(NW) * WIN)
    wend = np.searchsorted(ed, np.arange(1, NW + 1) * WIN)
    for w in range(NW):
        seg_src = es[wstart[w]:wend[w]]
        seg_dst = ed[wstart[w]:wend[w]] - w * WIN
        cnt = len(seg_src)
        assert cnt <= B * 128
        base = w * B * 128
        src_arr[base:base + cnt] = seg_src
        i = np.arange(cnt)
        S[w, i % 128, (i // 128) * 128 + seg_dst] = 1.0
        ST[w, seg_dst, (i // 128) * 128 + (i % 128)] = 1.0
    return src_arr, S, ST


def _max_blocks(src, dst):
    best = 0
    for m in range(NCORES):
        lo = m * CHUNK
        sel = (dst >= lo) & (dst < lo + CHUNK)
        ed = dst[sel] - lo
        cnt = np.bincount(ed // WIN, minlength=NW)
        best = max(best, int(np.max((cnt + 127) // 128)))
    return best


def _emit_rsqrt(nc, pool, vcol, P, ncols, coeffs, tag, eng=None):
    """rs = rsqrt(vcol + EPS) via quadratic + 1 Newton step."""
    e = eng if eng is not None else nc.vector
    Q2, Q1, Q0 = coeffs
    rs = pool.tile([128, ncols], F32, tag=f"rs{tag}")
    e.tensor_scalar(rs[:P], vcol, Q2, Q1, ALU.mult, ALU.add)
    e.tensor_mul(rs[:P], rs[:P], vcol)
    e.tensor_scalar(rs[:P], rs[:P], Q0, None, ALU.add)
    vep = pool.tile([128, ncols], F32, tag=f"vep{tag}")
    e.tensor_scalar(vep[:P], vcol, EPS, None, ALU.add)
    t_ = pool.tile([128, ncols], F32, tag=f"tn{tag}")
    e.tensor_mul(t_[:P], rs[:P], rs[:P])
    e.tensor_mul(t_[:P], t_[:P], vep[:P])
    e.tensor_scalar(t_[:P], t_[:P], -0.5, 1.5, ALU.mult, ALU.add)
    e.tensor_mul(rs[:P], rs[:P], t_[:P])
    return rs


def _build_program(B, phases=3):
    nc = bacc.Bacc("TRN2", target_bir_lowering=False, debug=False,
                   num_devices=NCORES)
    BL = B * 128               # padded edges per (window, rel)
    BL2 = 2 * BL
    W16 = BL2 // 16            # idx cols per window (both rels)
    L16 = NW * W16
    RC1 = _rsqrt_coeffs(*_BN1_RANGE)
    RC2 = _rsqrt_coeffs(*_BN2_RANGE)

    # ---- DRAM tensors (all per-core node data ROTATED by -lo mod N) ----
    x_bf = nc.dram_tensor("x_bf", [N, T * D], BF16, kind="ExternalInput")
    xc = nc.dram_tensor("xc", [CHUNK, T * D], BF16, kind="ExternalInput")
    bn1_gb = nc.dram_tensor("bn1_gb", [N, 2], F32, kind="ExternalInput")
    bn2_gb = nc.dram_tensor("bn2_gb", [CHUNK, 2], F32, kind="ExternalInput")
    w_in, al_in, ar_in, s_in, st_in = [], [], [], [], []
    for r in (1, 2):
        w_in.append(nc.dram_tensor(f"W{r}", [D, H * DH], F32, kind="ExternalInput"))
        al_in.append(nc.dram_tensor(f"al{r}t", [D, H * DH], F32, kind="ExternalInput"))
        ar_in.append(nc.dram_tensor(f"ar{r}t", [D, H * DH], F32, kind="ExternalInput"))
        s_in.append(nc.dram_tensor(f"S{r}", [NW, 128, BL], FP8, kind="ExternalInput"))
        st_in.append(nc.dram_tensor(f"ST{r}", [NW, 128, BL], FP8, kind="ExternalInput"))
    si_in = nc.dram_tensor("srcidx", [128, L16], I16, kind="ExternalInput")
    ffw1_in = nc.dram_tensor("ffw1", [D, DFF], F32, kind="ExternalInput")
    ffb1_in = nc.dram_tensor("ffb1", [DFF, 1], F32, kind="ExternalInput")
    ffw2_in = nc.dram_tensor("ffw2", [DFF, D], F32, kind="ExternalInput")
    ffb2_in = nc.dram_tensor("ffb2", [D, 1], F32, kind="ExternalInput")
    ident_in = nc.dram_tensor("ident", [128, 128], BF16, kind="ExternalInput")
    out_d = nc.dram_tensor("OUT", [CHUNK, T * D], BF16, kind="ExternalOutput")

    # interleaved: row 2*rot(node) + rel, 1024B each
    zpackB = nc.dram_tensor("zpackB", [2 * N, ZROW], BF16, kind="Internal")

    with tile.TileContext(nc) as tc:
        with (
            tc.tile_pool(name="const", bufs=1) as cpool,
            tc.tile_pool(name="zg", bufs=PREPD) as zgp,
        ):
            # ---- Phase 0 ----
            ident = cpool.tile([128, 128], BF16)
            nc.sync.dma_start(ident[:], ident_in[:])
            # wpair: fp8, cols [r(2), par(2), 68(=64 z + 4 el)] then er cols
            # 272:288 laid out [par(2), r(2), H] (block-diagonal over par).
            wpair = cpool.tile([128, 288], BF16)
            nc.vector.memset(wpair[:], 0.0)
            for r in range(2):
                wf = cpool.tile([D, H * DH], F32, tag="wf")
                nc.sync.dma_start(wf[:], w_in[r][:])
                for par in range(2):
                    dst = wpair[par * D:(par + 1) * D, :272].rearrange(
                        "p (r q c) -> p r q c", r=2, q=2)
                    nc.vector.tensor_copy(dst[:, r, par, 0:64], wf[:])
                for which, t_in in (("al", al_in[r]), ("ar", ar_in[r])):
                    alt = cpool.tile([D, H * DH], F32, tag="alt")
                    nc.sync.dma_start(alt[:], t_in[:])
                    prod = cpool.tile([D, H * DH], F32, tag="prod")
                    nc.vector.tensor_mul(prod[:], wf[:], alt[:])
                    red = cpool.tile([D, H], F32, tag="red")
                    nc.vector.tensor_reduce(
                        red[:].unsqueeze(2),
                        prod[:].rearrange("p (h k) -> p h k", k=DH),
                        mybir.AxisListType.X, ALU.add)
                    for par in range(2):
                        if which == "al":
                            dst = wpair[par * D:(par + 1) * D, :272].rearrange(
                                "p (r q c) -> p r q c", r=2, q=2)
                            nc.vector.tensor_copy(dst[:, r, par, 64:68], red[:])
                        else:
                            dst = wpair[par * D:(par + 1) * D, 272:288] \
                                .rearrange("p (q r h) -> p q r h", q=2, r=2)
                            nc.vector.tensor_copy(dst[:, par, r, :], red[:])
            ffw1 = cpool.tile([128, DFF], BF16)
            t1 = cpool.tile([D, DFF], F32, tag="t1")
            nc.sync.dma_start(t1[:], ffw1_in[:])
            nc.vector.tensor_copy(ffw1[0:D, :], t1[:])
            nc.sync.dma_start(ffw1[64:128, :], ffw1[0:64, :])
            ffw2 = cpool.tile([DFF, D], BF16)
            t2 = cpool.tile([DFF, D], F32, tag="t2")
            nc.sync.dma_start(t2[:], ffw2_in[:])
            nc.vector.tensor_copy(ffw2[:], t2[:])
            ffb1 = cpool.tile([DFF, 1], F32)
            nc.sync.dma_start(ffb1[:], ffb1_in[:])
            ffb2r = cpool.tile([128, 1], F32)
            nc.sync.dma_start(ffb2r[0:64, :], ffb2_in[:])
            nc.sync.dma_start(ffb2r[64:128, :], ffb2_in[:])
            # per-window er stash: [128, NW, r, T, H] fp8 (tiny)
            erw_all = cpool.tile([128, NW, 2, T, H], FP8)
            nc.vector.memset(erw_all[:], 0.0)

            zgs = {}

            # ---- Phase 1: BN1 + projections for all N nodes (rotated) ----
            with (
                tc.tile_pool(name="p1x", bufs=2) as p1x,
                tc.tile_pool(name="p1z", bufs=3) as p1z,
                tc.tile_pool(name="p1h", bufs=3) as p1h,
                tc.tile_pool(name="p1s", bufs=3) as p1s,
                tc.tile_pool(name="p1tp", bufs=2, space="PSUM") as p1tp,
                tc.tile_pool(name="p1zp", bufs=1, space="PSUM") as p1zp,
            ):
                def stage_a(sb):
                    # batched x/gb DMA + stats + rsqrt poly for one super
                    nsb = min(SUP, NBLK - sb)
                    n0 = sb * 128
                    nn = min(SUP * 128, N - n0)
                    full = (nn == nsb * 128)
                    xt4 = p1x.tile([128, SUP, T * D], BF16, tag="xt4")
                    gbt = p1s.tile([128, SUP, 2], F32, tag="gbt")
                    if full:
                        nc.sync.dma_start(
                            xt4[:, 0:nsb, :],
                            x_bf[n0:n0 + nn].rearrange(
                                "(j p) c -> p j c", p=128))
                        nc.sync.dma_start(
                            gbt[:, 0:nsb, :],
                            bn1_gb[n0:n0 + nn].rearrange(
                                "(j p) c -> p j c", p=128))
                    mvt = p1s.tile([128, SUP, 2], F32, tag="mvt")
                    for j in range(nsb):
                        nb = min(128, N - (sb + j) * 128)
                        if not full:
                            nc.sync.dma_start(
                                xt4[:nb, j, :],
                                x_bf[(sb + j) * 128:(sb + j) * 128 + nb])
                            nc.sync.dma_start(
                                gbt[:nb, j, :],
                                bn1_gb[(sb + j) * 128:(sb + j) * 128 + nb])
                        st6 = p1s.tile([128, 2, 6], F32, tag="st6")
                        nc.vector.bn_stats(st6[:nb, 0, :], xt4[:nb, j, 0:384])
                        nc.vector.bn_stats(st6[:nb, 1, :],
                                           xt4[:nb, j, 384:768])
                        nc.vector.bn_aggr(mvt[:nb, j, :], st6[:nb])
                    rs = _emit_rsqrt(nc, p1s, mvt[:, 0:nsb, 1], 128, nsb,
                                     RC1, "p1", eng=nc.gpsimd)
                    ab = p1s.tile([128, SUP, 2], F32, tag="ab")
                    nc.gpsimd.tensor_mul(ab[:, 0:nsb, 0], gbt[:, 0:nsb, 0],
                                         rs[:, 0:nsb])
                    nc.gpsimd.tensor_mul(ab[:, 0:nsb, 1], ab[:, 0:nsb, 0],
                                         mvt[:, 0:nsb, 0])
                    nc.gpsimd.tensor_sub(ab[:, 0:nsb, 1], gbt[:, 0:nsb, 1],
                                         ab[:, 0:nsb, 1])
                    return xt4, ab, nsb

                for sb in range(0, NBLK, SUP):
                    xt4, ab, nsb = stage_a(sb)
                    for j in range(nsb):
                        blk = sb + j
                        own = blk < NW   # own dst window (er stash)
                        ncol = 288 if own else 272
                        nb = min(128, N - blk * 128)
                        h = p1h.tile([128, T * D], BF16, tag="h")
                        nc.gpsimd.tensor_scalar(
                            h[:nb], xt4[:nb, j, :], ab[:nb, j, 0:1],
                            ab[:nb, j, 1:2], ALU.mult, ALU.add)
                        tp = p1tp.tile([128, NPAIR, 128], BF16, tag="tp")
                        for p in range(NPAIR):
                            nc.tensor.transpose(
                                tp[:, p, 0:nb], h[:nb, p * 128:(p + 1) * 128],
                                ident[:nb, :nb])
                        ht = p1h.tile([128, NPAIR, 128], BF16, tag="ht")
                        nc.scalar.activation(ht[:, :, 0:nb],
                                             tp[:, :, 0:nb], AF.Copy)
                        zel = p1z.tile([128, 2, T * 68], BF16, tag="zel")
                        HP = NPAIR // 2
                        for half in range(2):
                            q0 = half * HP
                            # one PSUM bank (512 f32) per pair; 3 banks/half
                            zp = p1zp.tile([128, HP, 512], F32,
                                           tag=f"zp{half}", bufs=1)
                            for p in range(HP):
                                nc.tensor.matmul(
                                    zp[0:nb, p, 0:ncol],
                                    ht[:, q0 + p, 0:nb],
                                    wpair[:, 0:ncol],
                                    start=True, stop=True)
                            zp_v = zp[:nb, :, 0:272].rearrange(
                                "p q (r par c) -> p q r par c", r=2, c=68)
                            for r in range(2):
                                dst_ap = zel[:nb, r,
                                             q0 * 136:(q0 + HP) * 136] \
                                    .rearrange("p (q par c) -> p q par c",
                                               q=HP, c=68)
                                if r == 0:
                                    nc.scalar.activation(
                                        dst_ap, zp_v[:, :, r], AF.Copy)
                                else:
                                    nc.vector.tensor_copy(
                                        dst_ap, zp_v[:, :, r])
                            if own:
                                nw = _win_nodes(blk)
                                nc.vector.tensor_copy(
                                    erw_all[:nw, blk, :,
                                            q0 * 2:(q0 + HP) * 2, :]
                                    .rearrange("p r (q par) h -> p q par r h",
                                               par=2),
                                    zp[:nw, :, 272:288].rearrange(
                                        "p q (par r h) -> p q par r h",
                                        par=2, r=2))
                        nc.sync.dma_start(
                            zpackB[2 * blk * 128:2 * blk * 128 + 2 * nb,
                                   0:T * 68],
                            zel[:nb].rearrange("p r c -> p (r c)"))

            # ---- Phase 2: fused gather/attention/segment-sum/BN2/FFN ----
            # Software-pipelined: the attention front-end (ebc, lk, leaky,
            # exp broadcast, denominator exp) of window w+1 is emitted in
            # iteration w, overlapping the back-end (mz, segment-sum,
            # epilogue, BN2, FFN) of window w.
            with (
                tc.tile_pool(name="x2p", bufs=2) as x2p,
                tc.tile_pool(name="abp", bufs=2) as abp,
                tc.tile_pool(name="sp", bufs=2) as spp,
                tc.tile_pool(name="msg", bufs=2) as msgp,
                tc.tile_pool(name="p2s", bufs=3) as p2s,
                tc.tile_pool(name="p2t", bufs=2) as p2t,
                tc.tile_pool(name="pp", bufs=1, space="PSUM") as pp,
            ):
                pre_f, pre_b, msgbs = {}, {}, {}

                def prefetch_stb(w):
                    ss = []
                    for r in range(2):
                        stb = spp.tile([128, BL], FP8, tag=f"stb{r}", bufs=2)
                        nc.sync.dma_start(stb[:], st_in[r][w])
                        ss.append(stb)
                    pre_f[w] = ss

                sis = {}

                def prefetch_si(w):
                    si = spp.tile([128, W16], I16, tag="si", bufs=4)
                    nc.sync.dma_start(si[:], si_in[:, w * W16:(w + 1) * W16])
                    sis[w] = si

                def prefetch_back(w):
                    nw = _win_nodes(w)
                    xcw = p2t.tile([128, T * D], BF16, tag="xcw")
                    nc.sync.dma_start(xcw[:nw], xc[w * WIN:w * WIN + nw])
                    gb2 = p2s.tile([128, 2], F32, tag="gb2", bufs=2)
                    nc.sync.dma_start(gb2[:nw], bn2_gb[w * WIN:w * WIN + nw])
                    ss = []
                    for r in range(2):
                        ssb = spp.tile([128, BL], FP8, tag=f"ssb{r}")
                        nc.sync.dma_start(ssb[:], s_in[r][w])
                        ss.append(ssb)
                    pre_b[w] = (xcw, gb2, ss)

                def issue_gather(w):
                    zg = zgp.tile([128, 2 * B, ZROW], BF16, tag="zg")
                    si = sis.pop(w)
                    nc.gpsimd.dma_gather(
                        zg[:], zpackB[:], si[:],
                        BL2, BL2, ZROW, single_packet=False)
                    zgs[w] = zg

                lks = {}

                def front_pre(w):
                    # ebc + lk + leaky for window w (emitted one early)
                    zg = zgs[w]
                    stbs = pre_f.pop(w)
                    for r in range(2):
                        stb = stbs[r]
                        lk = p2s.tile([128, B, T * H], BF16, tag="lk",
                                      bufs=2)
                        ebc = pp.tile([128, B, 64], F32, tag="ebc")
                        for b in range(B):
                            nc.tensor.matmul(
                                ebc[:, b, 0:T * H],
                                stb[:, b * 128:(b + 1) * 128],
                                erw_all[:, w, r].rearrange(
                                    "p q h -> p (q h)"),
                                start=True, stop=True)
                        el_ap = zg[:, r * B:(r + 1) * B, 0:T * 68] \
                            .rearrange("p b (t c) -> p b t c",
                                       c=68)[:, :, :, 64:68]
                        nc.vector.tensor_add(
                            lk[:].rearrange("p b (t h) -> p b t h", h=H),
                            el_ap,
                            ebc[:, :, 0:T * H].rearrange(
                                "p b (t h) -> p b t h", h=H))
                        nc.vector.scalar_tensor_tensor(
                            lk[:], lk[:], NEG_SLOPE, lk[:], ALU.mult,
                            ALU.max)
                        lks[(w, r)] = lk

                def front_exp(w):
                    for r in range(2):
                        lk = lks.pop((w, r))
                        msgb = msgp.tile([128, B, 816], BF16,
                                         tag=f"msg{r}", bufs=1)
                        nc.scalar.activation(
                            msgb[:, :, 0:768].rearrange(
                                "p b (t h k) -> p b t h k", h=H, k=DH),
                            lk[:].rearrange("p b (t h) -> p b t h", h=H)
                            .unsqueeze(4).broadcast_to((128, B, T, H, DH)),
                            AF.Exp)
                        nc.scalar.activation(msgb[:, :, 768:816], lk[:],
                                             AF.Exp)
                        msgbs[(w, r)] = msgb

                prefetch_stb(0)
                prefetch_back(0)
                for w in range(min(PREPD, NW)):
                    prefetch_si(w)
                    issue_gather(w)
                for w in range(NW):
                    nw = _win_nodes(w)
                    if w + PREPD < NW:
                        prefetch_si(w + PREPD)
                        issue_gather(w + PREPD)
                    if w + 1 < NW:
                        prefetch_stb(w + 1)
                        prefetch_back(w + 1)
                    xcw, gb2, ssbs = pre_b.pop(w)
                    front_pre(w)
                    front_exp(w)
                    zg = zgs.pop(w)
                    msgbw = []
                    for r in range(2):
                        zap = zg[:, r * B:(r + 1) * B, 0:T * 68].rearrange(
                            "p b (t c) -> p b t c", c=68)[:, :, :, 0:64] \
                            .rearrange("p b t (h k) -> p b t h k", k=DH)
                        msgb = msgbs.pop((w, r))
                        mz = msgb[:, :, 0:768].rearrange(
                            "p b (t h k) -> p b t h k", h=H, k=DH)
                        nc.vector.tensor_mul(mz, mz, zap)
                        msgbw.append(msgb)
                    msum = []
                    for r in range(2):
                        msgb = msgbw[r]
                        ssb = ssbs[r]
                        ms = pp.tile([128, 816], F32, tag="big", bufs=2)
                        for b in range(B):
                            lhsT = ssb[:, b * 128:(b + 1) * 128]
                            nc.tensor.matmul(ms[:, 0:512], lhsT,
                                             msgb[:, b, 0:512],
                                             start=(b == 0),
                                             stop=(b == B - 1))
                            nc.tensor.matmul(ms[:, 512:816], lhsT,
                                             msgb[:, b, 512:816],
                                             start=(b == 0),
                                             stop=(b == B - 1))
                        msum.append(ms)
                    # epilogue: m = msgsum/denom; x2 = bf16(x + m1 + m2)
                    x2w = x2p.tile([128, T * D], BF16, tag="x2")
                    mtmp = p2t.tile([128, T * D], BF16, tag="mtmp")
                    for r in range(2):
                        rec = p2s.tile([128, T * H], F32, tag="rec")
                        nc.vector.tensor_scalar_max(
                            rec[:nw], msum[r][:nw, 768:816], 1e-16)
                        nc.vector.reciprocal(rec[:nw], rec[:nw])
                        rb = rec[:nw].rearrange(
                            "p (t h) -> p t h", h=H).unsqueeze(3) \
                            .broadcast_to((nw, T, H, DH))
                        dst = (mtmp if r == 0 else x2w)
                        nc.vector.tensor_mul(
                            dst[:nw].rearrange(
                                "p (t h k) -> p t h k", h=H, k=DH),
                            msum[r][:nw, 0:768].rearrange(
                                "p (t h k) -> p t h k", h=H, k=DH), rb)
                    nc.vector.tensor_add(mtmp[:nw], mtmp[:nw], xcw[:nw])
                    nc.vector.tensor_add(x2w[:nw], x2w[:nw], mtmp[:nw])
                    # BN2 stats + a2/b2
                    st6b = p2s.tile([128, 2, 6], F32, tag="st6b")
                    nc.vector.bn_stats(st6b[:nw, 0, :], x2w[:nw, 0:384])
                    nc.vector.bn_stats(st6b[:nw, 1, :], x2w[:nw, 384:768])
                    mvb = p2s.tile([128, 2], F32, tag="mvb")
                    nc.vector.bn_aggr(mvb[:nw], st6b[:nw])
                    rs2 = _emit_rsqrt(nc, p2s, mvb[:nw, 1:2], nw, 1, RC2,
                                      "b2")
                    ab2 = abp.tile([128, 2], F32, tag="ab2")
                    nc.vector.tensor_mul(ab2[:nw, 0:1], gb2[:nw, 0:1],
                                         rs2[:nw])
                    nc.vector.tensor_mul(ab2[:nw, 1:2], ab2[:nw, 0:1],
                                         mvb[:nw, 0:1])
                    nc.vector.tensor_sub(ab2[:nw, 1:2], gb2[:nw, 1:2],
                                         ab2[:nw, 1:2])
                    if phases < 3:
                        xo = p2t.tile([128, T * D], BF16, tag="mtmp")
                        nc.vector.tensor_copy(xo[:nw], x2w[:nw])
                        nc.sync.dma_start(out_d[w * WIN:w * WIN + nw],
                                          xo[:nw])
                        continue
                    # ---- FFN: BN2 apply + 2 layers + residual ----
                    h2 = p2t.tile([128, T * D], BF16, tag="hw")
                    nc.scalar.activation(h2[:nw], x2w[:nw], AF.Identity,
                                         bias=ab2[:nw, 1:2],
                                         scale=ab2[:nw, 0:1])
                    h2t = p2t.tile([64, T, 128], BF16, tag="h2t")
                    for half in range(2):
                        tp = pp.tile([64, NPAIR, 128], BF16, tag="tp3")
                        for j in range(NPAIR):
                            t = half * NPAIR + j
                            nc.tensor.transpose(
                                tp[:, j, 0:nw], h2[:nw, t * 64:(t + 1) * 64],
                                ident[:nw, :nw])
                        nc.scalar.activation(
                            h2t[:, half * NPAIR:(half + 1) * NPAIR, 0:nw],
                            tp[:, :, 0:nw], AF.Copy)
                    if nw < 128:
                        nc.vector.memset(h2t[:, :, nw:128], 0.0)
                    dd = pp.tile([128, T, 64], BF16, tag="tpS")
                    fft = p2t.tile([64, T, 128], BF16, tag="fft")
                    for k in range(3):
                        big = pp.tile([128, 816], F32, tag="big", bufs=2)
                        f1 = big[:, 0:512]
                        rhs = h2t[:, 4 * k:4 * k + 4, :]
                        nc.tensor.matmul(f1, ffw1[0:64, :],
                                         rhs.rearrange("p a b -> p (a b)"),
                                         start=True, stop=True)
                        g1 = p2t.tile([128, 512], BF16, tag="g1")
                        nc.scalar.activation(g1[:], f1, AF.Gelu,
                                             bias=ffb1[:])
                        for half in range(2):
                            f2 = big[0:64, 512:768]
                            nc.tensor.matmul(
                                f2, ffw2[:],
                                g1[:, half * 256:(half + 1) * 256],
                                start=True, stop=True)
                            nc.vector.tensor_scalar(
                                fft[:, 4 * k + 2 * half:
                                    4 * k + 2 * half + 2, :]
                                .rearrange("p a b -> p (a b)"),
                                f2, ffb2r[0:64, :], None, ALU.add)
                    for t in range(T):
                        nc.tensor.transpose(
                            dd[0:nw, t, :], fft[:, t, 0:nw],
                            ident[0:64, 0:64])
                    ot = p2t.tile([128, T * D], BF16, tag="mtmp")
                    nc.vector.tensor_add(
                        ot[:nw], dd[:nw].rearrange("p a b -> p (a b)"),
                        x2w[:nw])
                    nc.sync.dma_start(out_d[w * WIN:w * WIN + nw], ot[:nw])

    nc.compile()
    return nc


_CACHE = {}
_PHASES = 3
_TRACE = False
_TRACE_DIR = None
_LAST_EXEC_NS = None


def _host_prep(inputs):
    x = np.asarray(inputs["x"], np.float32)
    xf = np.ascontiguousarray(x.reshape(N, T * D))
    xbf_full = xf.astype(BF16NP)
    B = 0
    for r in (1, 2):
        B = max(B, _max_blocks(np.asarray(inputs[f"src{r}"]),
                               np.asarray(inputs[f"dst{r}"])))

    bn1_gb_full = np.ascontiguousarray(
        np.stack([np.asarray(inputs["bn1_g"], np.float32),
                  np.asarray(inputs["bn1_b"], np.float32)], axis=1))
    bn2_gb_full = np.ascontiguousarray(
        np.stack([np.asarray(inputs["bn2_g"], np.float32),
                  np.asarray(inputs["bn2_b"], np.float32)], axis=1))
    common = {
        "ffw1": np.ascontiguousarray(np.asarray(inputs["ff_w1"], np.float32)),
        "ffb1": np.ascontiguousarray(
            np.asarray(inputs["ff_b1"], np.float32).reshape(DFF, 1)),
        "ffw2": np.ascontiguousarray(np.asarray(inputs["ff_w2"], np.float32)),
        "ffb2": np.ascontiguousarray(
            np.asarray(inputs["ff_b2"], np.float32).reshape(D, 1)),
        "ident": np.eye(128, dtype=BF16NP),
    }
    for r in (1, 2):
        W = np.asarray(inputs[f"W{r}"], np.float32).reshape(D, H * DH)
        al = np.asarray(inputs[f"al{r}"], np.float32).reshape(-1)
        ar = np.asarray(inputs[f"ar{r}"], np.float32).reshape(-1)
        common[f"W{r}"] = np.ascontiguousarray(W)
        common[f"al{r}t"] = np.ascontiguousarray(np.tile(al[None, :], (D, 1)))
        common[f"ar{r}t"] = np.ascontiguousarray(np.tile(ar[None, :], (D, 1)))

    BL = B * 128
    in_maps = []
    for m in range(NCORES):
        lo = m * CHUNK
        im = dict(common)
        # rotated per-core node ordering: row i = node (lo + i) mod N
        im["x_bf"] = np.ascontiguousarray(np.roll(xbf_full, -lo, axis=0))
        im["bn1_gb"] = np.ascontiguousarray(np.roll(bn1_gb_full, -lo, axis=0))
        im["xc"] = np.ascontiguousarray(xbf_full[lo:lo + CHUNK])
        im["bn2_gb"] = np.ascontiguousarray(bn2_gb_full[lo:lo + CHUNK])
        srcs = []
        for r in (1, 2):
            src_flat, S, ST = _prep_core_rel(
                np.asarray(inputs[f"src{r}"]), np.asarray(inputs[f"dst{r}"]),
                lo, B)
            im[f"S{r}"] = S
            im[f"ST{r}"] = ST
            srcs.append(2 * src_flat + (r - 1))  # interleaved zpackB rows
        idx = np.empty((128, NW * (2 * BL) // 16), np.int16)
        for w in range(NW):
            for r in range(2):
                seg = srcs[r][w * BL:(w + 1) * BL].astype(np.int16)
                col0 = (w * 2 + r) * (BL // 16)
                idx[:, col0:col0 + BL // 16] = np.tile(
                    seg.reshape(-1, 16).T, (8, 1))
        im["srcidx"] = np.ascontiguousarray(idx)
        in_maps.append(im)
    return B, in_maps


def kernel(**inputs):
    B, in_maps = _host_prep(inputs)
    key = (B, _PHASES)
    if key not in _CACHE:
        _CACHE[key] = _build_program(B, _PHASES)
    nc = _CACHE[key]
    global _LAST_EXEC_NS
    tmpdir = None
    if _TRACE and _TRACE_DIR:
        import os, shutil
        shutil.rmtree(_TRACE_DIR, ignore_errors=True)
        os.makedirs(_TRACE_DIR, exist_ok=True)
        tmpdir = _TRACE_DIR
    res = run_bass_kernel_spmd(nc, in_maps, core_ids=list(range(NCORES)),
                               trace=_TRACE, tmpdir=tmpdir)
    _LAST_EXEC_NS = res.exec_time_ns
    out = np.concatenate([res.results[m]["OUT"] for m in range(NCORES)],
                         axis=0)
    return out.reshape(N, T, D).astype(np.float32)


# revision 42
# speedup vs baseline: 1.1783x; 1.0138x over previous
"""Trainium2 Bass kernel for nn_EncoderLayer (GNN message passing, 2-relation GAT).

Sharding: nodes (and incoming-edge lists, partitioned by dst) sharded across 8
cores; small GAT/FFN weights replicated; gathered src features fetched from a
replicated projection table via indexed DMA (dma_gather).

v8 layout (per core, node ids ROTATED so own chunk = rows [0, CHUNK)):
  Phase 0: fold weights (bf16 wpair with the er columns appended at 272:288).
  Phase 1: BN1 (bf16 x input; stats on vector, rsqrt poly + affine on the
           otherwise-idle gpsimd; h = a*x+b on gpsimd) + z/el projection for
           ALL N nodes via bf16 transposes + matmuls split into two 3-pair
           PSUM halves so the next half's matmuls overlap the PSUM->SBUF
           copies (zel r0 on scalar, r1 on vector). Packed bf16 rows
           zpackB[2*rot(node) + rel] = 12 x (64 z | 4 el), 1792B stride.
           Blocks 0..9 are this core's own dst windows: the er columns are
           stashed on-chip in erw_all (fp8, tiny).
  Phase 2 (per dst-window, in-window schedule -- cross-window pipelining
           loses to conservative cross-engine tick waits on this scheduler):
           plain gathers with a 3-deep zg ring, issued at iteration start;
           er broadcast edge-wise via fp8 ST matmuls into a bank-aligned
           [B,64] PSUM tile; single lk add + leaky per rel; exp broadcast
           over dh by scalar into msgb (single-buffered per rel); mz
           multiplied IN PLACE in msgb on vector (keeps zg's last reader on
           vector so the next gather's WAR resolves early); segment-sum via
           fp8 S x bf16 msgb matmuls (768 msg + 48 denom cols); epilogue
           m = msgsum/denom, x2 = x + m1 + m2 (bf16), BN2, FFN, bf16 output
           (host converts to f32).
"""

import sys

sys.path.insert(0, "/opt/trn_rl_repo")

import numpy as np
import ml_dtypes

import concourse.bass as bass
import concourse.bacc as bacc
import concourse.tile as tile
import concourse.mybir as mybir
from concourse.bass_utils import run_bass_kernel_spmd

F32 = mybir.dt.float32
BF16 = mybir.dt.bfloat16
FP8 = mybir.dt.float8e4
I16 = mybir.dt.int16
AF = mybir.ActivationFunctionType
ALU = mybir.AluOpType
BF16NP = ml_dtypes.bfloat16
F8NP = ml_dtypes.float8_e4m3

N, T, D, H, DH, DFF = 10000, 12, 64, 4, 16, 128
NCORES = 8
CHUNK = N // NCORES          # 1250
WIN = 128                    # dst-window size (nodes)
NW = (CHUNK + WIN - 1) // WIN  # 10 windows; last has 98 nodes
EPS = 1e-5
NEG_SLOPE = 0.2
ZROW = 896                   # zpack row (bf16 elems): 12*68 data + 80 pad
NBLK = (N + 127) // 128      # 79 phase-1 blocks (last = 16 nodes)
NPAIR = T // 2               # 6 paired (2-timestep) transposes per block
SUP = 8                      # phase-1 super-block (batched DMA + rsqrt math)
PREPD = 3                    # gather ring depth (zg bufs)

# rsqrt(v + EPS) = quadratic fit + one Newton step (vector engine only).
_BN1_RANGE = (0.55, 1.6)
_BN2_RANGE = (0.55, 3.2)


def _rsqrt_coeffs(lo, hi):
    v = np.linspace(lo, hi, 4001)
    c = np.polyfit(v, 1.0 / np.sqrt(v + EPS), 2)
    return [float(x) for x in c]


def _win_nodes(w):
    return min(WIN, CHUNK - w * WIN)


def _prep_core_rel(src, dst, lo, B):
    """Edge lists for one (core, relation): sorted by dst, windowed, padded
    to B blocks of 128 edges per window. Node ids are ROTATED by -lo mod N.
    Returns (src_flat[NW*B*128], S fp8, ST fp8)."""
    hi = lo + CHUNK
    sel = (dst >= lo) & (dst < hi)
    es = ((src[sel].astype(np.int64) - lo) % N)    # rotated src ids
    ed = (dst[sel] - lo).astype(np.int64)
    order = np.argsort(ed, kind="stable")
    es, ed = es[order], ed[order]
    L = NW * B * 128
    src_arr = np.zeros(L, np.int64)
    S = np.zeros((NW, 128, B * 128), F8NP)
    ST = np.zeros((NW, 128, B * 128), F8NP)
    wstart = np.searchsorted(ed, np.arange(NW) * WIN)
    wend = np.searchsorted(ed, np.arange(1, NW + 1) * WIN)
    for w in range(NW):
        seg_src = es[wstart[w]:wend[w]]
        seg_dst = ed[wstart[w]:wend[w]] - w * WIN
        cnt = len(seg_src)
        assert cnt <= B * 128
        base = w * B * 128
        src_arr[base:base + cnt] = seg_src
        i = np.arange(cnt)
        S[w, i % 128, (i // 128) * 128 + seg_dst] = 1.0
        ST[w, seg_dst, (i // 128) * 128 + (i % 128)] = 1.0
    return src_arr, S, ST


def _max_blocks(src, dst):
    best = 0
    for m in range(NCORES):
        lo = m * CHUNK
        sel = (dst >= lo) & (dst < lo + CHUNK)
        ed = dst[sel] - lo
        cnt = np.bincount(ed // WIN, minlength=NW)
        best = max(best, int(np.max((cnt + 127) // 128)))
    return best


def _emit_rsqrt(nc, pool, vcol, P, ncols, coeffs, tag, eng=None):
    """rs = rsqrt(vcol + EPS) via quadratic + 1 Newton step."""
    e = eng if eng is not None else nc.vector
    Q2, Q1, Q0 = coeffs
    rs = pool.tile([128, ncols], F32, tag=f"rs{tag}")
    e.tensor_scalar(rs[:P], vcol, Q2, Q1, ALU.mult, ALU.add)
    e.tensor_mul(rs[:P], rs[:P], vcol)
    e.tensor_scalar(rs[:P], rs[:P], Q0, None, ALU.add)
    vep = pool.tile([128, ncols], F32, tag=f"vep{tag}")
    e.tensor_scalar(vep[:P], vcol, EPS, None, ALU.add)
    t_ = pool.tile([128, ncols], F32, tag=f"tn{tag}")
    e.tensor_mul(t_[:P], rs[:P], rs[:P])
    e.tensor_mul(t_[:P], t_[:P], vep[:P])
    e.tensor_scalar(t_[:P], t_[:P], -0.5, 1.5, ALU.mult, ALU.add)
    e.tensor_mul(rs[:P], rs[:P], t_[:P])
    return rs


def _build_program(B, phases=3):
    nc = bacc.Bacc("TRN2", target_bir_lowering=False, debug=False,
                   num_devices=NCORES)
    BL = B * 128               # padded edges per (window, rel)
    BL2 = 2 * BL
    W16 = BL2 // 16            # idx cols per window (both rels)
    L16 = NW * W16
    RC1 = _rsqrt_coeffs(*_BN1_RANGE)
    RC2 = _rsqrt_coeffs(*_BN2_RANGE)

    # ---- DRAM tensors (all per-core node data ROTATED by -lo mod N) ----
    x_bf = nc.dram_tensor("x_bf", [N, T * D], BF16, kind="ExternalInput")
    xc = nc.dram_tensor("xc", [CHUNK, T * D], BF16, kind="ExternalInput")
    bn1_gb = nc.dram_tensor("bn1_gb", [N, 2], F32, kind="ExternalInput")
    bn2_gb = nc.dram_tensor("bn2_gb", [CHUNK, 2], F32, kind="ExternalInput")
    w_in, al_in, ar_in, s_in, st_in = [], [], [], [], []
    for r in (1, 2):
        w_in.append(nc.dram_tensor(f"W{r}", [D, H * DH], F32, kind="ExternalInput"))
        al_in.append(nc.dram_tensor(f"al{r}t", [D, H * DH], F32, kind="ExternalInput"))
        ar_in.append(nc.dram_tensor(f"ar{r}t", [D, H * DH], F32, kind="ExternalInput"))
        s_in.append(nc.dram_tensor(f"S{r}", [NW, 128, BL], FP8, kind="ExternalInput"))
        st_in.append(nc.dram_tensor(f"ST{r}", [NW, 128, BL], FP8, kind="ExternalInput"))
    si_in = nc.dram_tensor("srcidx", [128, L16], I16, kind="ExternalInput")
    ffw1_in = nc.dram_tensor("ffw1", [D, DFF], F32, kind="ExternalInput")
    ffb1_in = nc.dram_tensor("ffb1", [DFF, 1], F32, kind="ExternalInput")
    ffw2_in = nc.dram_tensor("ffw2", [DFF, D], F32, kind="ExternalInput")
    ffb2_in = nc.dram_tensor("ffb2", [D, 1], F32, kind="ExternalInput")
    ident_in = nc.dram_tensor("ident", [128, 128], BF16, kind="ExternalInput")
    out_d = nc.dram_tensor("OUT", [CHUNK, T * D], BF16, kind="ExternalOutput")

    # interleaved: row 2*rot(node) + rel, 1024B each
    zpackB = nc.dram_tensor("zpackB", [2 * N, ZROW], BF16, kind="Internal")

    with tile.TileContext(nc) as tc:
        with (
            tc.tile_pool(name="const", bufs=1) as cpool,
            tc.tile_pool(name="zg", bufs=PREPD) as zgp,
        ):
            # ---- Phase 0 ----
            ident = cpool.tile([128, 128], BF16)
            nc.sync.dma_start(ident[:], ident_in[:])
            # wpair: fp8, cols [r(2), par(2), 68(=64 z + 4 el)] then er cols
            # 272:288 laid out [par(2), r(2), H] (block-diagonal over par).
            wpair = cpool.tile([128, 288], BF16)
            nc.vector.memset(wpair[:], 0.0)
            for r in range(2):
                wf = cpool.tile([D, H * DH], F32, tag="wf")
                nc.sync.dma_start(wf[:], w_in[r][:])
                for par in range(2):
                    dst = wpair[par * D:(par + 1) * D, :272].rearrange(
                        "p (r q c) -> p r q c", r=2, q=2)
                    nc.vector.tensor_copy(dst[:, r, par, 0:64], wf[:])
                for which, t_in in (("al", al_in[r]), ("ar", ar_in[r])):
                    alt = cpool.tile([D, H * DH], F32, tag="alt")
                    nc.sync.dma_start(alt[:], t_in[:])
                    prod = cpool.tile([D, H * DH], F32, tag="prod")
                    nc.vector.tensor_mul(prod[:], wf[:], alt[:])
                    red = cpool.tile([D, H], F32, tag="red")
                    nc.vector.tensor_reduce(
                        red[:].unsqueeze(2),
                        prod[:].rearrange("p (h k) -> p h k", k=DH),
                        mybir.AxisListType.X, ALU.add)
                    for par in range(2):
                        if which == "al":
                            dst = wpair[par * D:(par + 1) * D, :272].rearrange(
                                "p (r q c) -> p r q c", r=2, q=2)
                            nc.vector.tensor_copy(dst[:, r, par, 64:68], red[:])
                        else:
                            dst = wpair[par * D:(par + 1) * D, 272:288] \
                                .rearrange("p (q r h) -> p q r h", q=2, r=2)
                            nc.vector.tensor_copy(dst[:, par, r, :], red[:])
            ffw1 = cpool.tile([128, DFF], BF16)
            t1 = cpool.tile([D, DFF], F32, tag="t1")
            nc.sync.dma_start(t1[:], ffw1_in[:])
            nc.vector.tensor_copy(ffw1[0:D, :], t1[:])
            nc.sync.dma_start(ffw1[64:128, :], ffw1[0:64, :])
            ffw2 = cpool.tile([DFF, D], BF16)
            t2 = cpool.tile([DFF, D], F32, tag="t2")
            nc.sync.dma_start(t2[:], ffw2_in[:])
            nc.vector.tensor_copy(ffw2[:], t2[:])
            ffb1 = cpool.tile([DFF, 1], F32)
            nc.sync.dma_start(ffb1[:], ffb1_in[:])
            ffb2r = cpool.tile([128, 1], F32)
            nc.sync.dma_start(ffb2r[0:64, :], ffb2_in[:])
            nc.sync.dma_start(ffb2r[64:128, :], ffb2_in[:])
            # per-window er stash: [128, NW, r, T, H] fp8 (tiny)
            erw_all = cpool.tile([128, NW, 2, T, H], FP8)
            nc.vector.memset(erw_all[:], 0.0)

            zgs = {}

            # ---- Phase 1: BN1 + projections for all N nodes (rotated) ----
            with (
                tc.tile_pool(name="p1x", bufs=2) as p1x,
                tc.tile_pool(name="p1z", bufs=3) as p1z,
                tc.tile_pool(name="p1h", bufs=3) as p1h,
                tc.tile_pool(name="p1s", bufs=3) as p1s,
                tc.tile_pool(name="p1tp", bufs=2, space="PSUM") as p1tp,
                tc.tile_pool(name="p1zp", bufs=1, space="PSUM") as p1zp,
            ):
                def stage_a(sb):
                    # batched x/gb DMA + stats + rsqrt poly for one super
                    nsb = min(SUP, NBLK - sb)
                    n0 = sb * 128
                    nn = min(SUP * 128, N - n0)
                    full = (nn == nsb * 128)
                    xt4 = p1x.tile([128, SUP, T * D], BF16, tag="xt4")
                    gbt = p1s.tile([128, SUP, 2], F32, tag="gbt")
                    if full:
                        nc.sync.dma_start(
                            xt4[:, 0:nsb, :],
                            x_bf[n0:n0 + nn].rearrange(
                                "(j p) c -> p j c", p=128))
                        nc.sync.dma_start(
                            gbt[:, 0:nsb, :],
                            bn1_gb[n0:n0 + nn].rearrange(
                                "(j p) c -> p j c", p=128))
                    mvt = p1s.tile([128, SUP, 2], F32, tag="mvt")
                    for j in range(nsb):
                        nb = min(128, N - (sb + j) * 128)
                        if not full:
                            nc.sync.dma_start(
                                xt4[:nb, j, :],
                                x_bf[(sb + j) * 128:(sb + j) * 128 + nb])
                            nc.sync.dma_start(
                                gbt[:nb, j, :],
                                bn1_gb[(sb + j) * 128:(sb + j) * 128 + nb])
                        st6 = p1s.tile([128, 2, 6], F32, tag="st6")
                        nc.vector.bn_stats(st6[:nb, 0, :], xt4[:nb, j, 0:384])
                        nc.vector.bn_stats(st6[:nb, 1, :],
                                           xt4[:nb, j, 384:768])
                        nc.vector.bn_aggr(mvt[:nb, j, :], st6[:nb])
                    rs = _emit_rsqrt(nc, p1s, mvt[:, 0:nsb, 1], 128, nsb,
                                     RC1, "p1", eng=nc.gpsimd)
                    ab = p1s.tile([128, SUP, 2], F32, tag="ab")
                    nc.gpsimd.tensor_mul(ab[:, 0:nsb, 0], gbt[:, 0:nsb, 0],
                                         rs[:, 0:nsb])
                    nc.gpsimd.tensor_mul(ab[:, 0:nsb, 1], ab[:, 0:nsb, 0],
                                         mvt[:, 0:nsb, 0])
                    nc.gpsimd.tensor_sub(ab[:, 0:nsb, 1], gbt[:, 0:nsb, 1],
                                         ab[:, 0:nsb, 1])
                    return xt4, ab, nsb

                for sb in range(0, NBLK, SUP):
                    xt4, ab, nsb = stage_a(sb)
                    for j in range(nsb):
                        blk = sb + j
                        own = blk < NW   # own dst window (er stash)
                        ncol = 288 if own else 272
                        nb = min(128, N - blk * 128)
                        h = p1h.tile([128, T * D], BF16, tag="h")
                        nc.gpsimd.tensor_scalar(
                            h[:nb], xt4[:nb, j, :], ab[:nb, j, 0:1],
                            ab[:nb, j, 1:2], ALU.mult, ALU.add)
                        tp = p1tp.tile([128, NPAIR, 128], BF16, tag="tp")
                        for p in range(NPAIR):
                            nc.tensor.transpose(
                                tp[:, p, 0:nb], h[:nb, p * 128:(p + 1) * 128],
                                ident[:nb, :nb])
                        ht = p1h.tile([128, NPAIR, 128], BF16, tag="ht")
                        nc.scalar.activation(ht[:, :, 0:nb],
                                             tp[:, :, 0:nb], AF.Copy)
                        zel = p1z.tile([128, 2, T * 68], BF16, tag="zel")
                        HP = NPAIR // 2
                        for half in range(2):
                            q0 = half * HP
                            # one PSUM bank (512 f32) per pair; 3 banks/half
                            zp = p1zp.tile([128, HP, 512], F32,
                                           tag=f"zp{half}", bufs=1)
                            for p in range(HP):
                                nc.tensor.matmul(
                                    zp[0:nb, p, 0:ncol],
                                    ht[:, q0 + p, 0:nb],
                                    wpair[:, 0:ncol],
                                    start=True, stop=True)
                            zp_v = zp[:nb, :, 0:272].rearrange(
                                "p q (r par c) -> p q r par c", r=2, c=68)
                            for r in range(2):
                                dst_ap = zel[:nb, r,
                                             q0 * 136:(q0 + HP) * 136] \
                                    .rearrange("p (q par c) -> p q par c",
                                               q=HP, c=68)
                                if r == 0:
                                    nc.scalar.activation(
                                        dst_ap, zp_v[:, :, r], AF.Copy)
                                else:
                                    nc.vector.tensor_copy(
                                        dst_ap, zp_v[:, :, r])
                            if own:
                                nw = _win_nodes(blk)
                                nc.vector.tensor_copy(
                                    erw_all[:nw, blk, :,
                                            q0 * 2:(q0 + HP) * 2, :]
                                    .rearrange("p r (q par) h -> p q par r h",
                                               par=2),
                                    zp[:nw, :, 272:288].rearrange(
                                        "p q (par r h) -> p q par r h",
                                        par=2, r=2))
                        nc.sync.dma_start(
                            zpackB[2 * blk * 128:2 * blk * 128 + 2 * nb,
                                   0:T * 68],
                            zel[:nb].rearrange("p r c -> p (r c)"))

            # ---- Phase 2: fused gather/attention/segment-sum/BN2/FFN ----
            # Software-pipelined: the attention front-end (ebc, lk, leaky,
            # exp broadcast, denominator exp) of window w+1 is emitted in
            # iteration w, overlapping the back-end (mz, segment-sum,
            # epilogue, BN2, FFN) of window w.
            with (
                tc.tile_pool(name="x2p", bufs=2) as x2p,
                tc.tile_pool(name="abp", bufs=2) as abp,
                tc.tile_pool(name="sp", bufs=2) as spp,
                tc.tile_pool(name="msg", bufs=2) as msgp,
                tc.tile_pool(name="p2s", bufs=3) as p2s,
                tc.tile_pool(name="p2t", bufs=2) as p2t,
                tc.tile_pool(name="pp", bufs=1, space="PSUM") as pp,
            ):
                pre_f, pre_b, msgbs = {}, {}, {}

                def prefetch_stb(w):
                    ss = []
                    for r in range(2):
                        stb = spp.tile([128, BL], FP8, tag=f"stb{r}", bufs=2)
                        nc.sync.dma_start(stb[:], st_in[r][w])
                        ss.append(stb)
                    pre_f[w] = ss

                sis = {}

                def prefetch_si(w):
                    si = spp.tile([128, W16], I16, tag="si", bufs=4)
                    nc.sync.dma_start(si[:], si_in[:, w * W16:(w + 1) * W16])
                    sis[w] = si

                def prefetch_back(w):
                    nw = _win_nodes(w)
                    xcw = p2t.tile([128, T * D], BF16, tag="xcw")
                    nc.sync.dma_start(xcw[:nw], xc[w * WIN:w * WIN + nw])
                    gb2 = p2s.tile([128, 2], F32, tag="gb2", bufs=2)
                    nc.sync.dma_start(gb2[:nw], bn2_gb[w * WIN:w * WIN + nw])
                    ss = []
                    for r in range(2):
                        ssb = spp.tile([128, BL], FP8, tag=f"ssb{r}")
                        nc.sync.dma_start(ssb[:], s_in[r][w])
                        ss.append(ssb)
                    pre_b[w] = (xcw, gb2, ss)

                def issue_gather(w):
                    zg = zgp.tile([128, 2 * B, ZROW], BF16, tag="zg")
                    si = sis.pop(w)
                    nc.gpsimd.dma_gather(
                        zg[:], zpackB[:], si[:],
                        BL2, BL2, ZROW, single_packet=False)
                    zgs[w] = zg

                lks = {}

                def front_pre(w):
                    # ebc + lk + leaky for window w (emitted one early)
                    zg = zgs[w]
                    stbs = pre_f.pop(w)
                    for r in range(2):
                        stb = stbs[r]
                        lk = p2s.tile([128, B, T * H], BF16, tag="lk",
                                      bufs=2)
                        ebc = pp.tile([128, B, 64], F32, tag="ebc")
                        for b in range(B):
                            nc.tensor.matmul(
                                ebc[:, b, 0:T * H],
                                stb[:, b * 128:(b + 1) * 128],
                                erw_all[:, w, r].rearrange(
                                    "p q h -> p (q h)"),
                                start=True, stop=True)
                        el_ap = zg[:, r * B:(r + 1) * B, 0:T * 68] \
                            .rearrange("p b (t c) -> p b t c",
                                       c=68)[:, :, :, 64:68]
                        nc.vector.tensor_add(
                            lk[:].rearrange("p b (t h) -> p b t h", h=H),
                            el_ap,
                            ebc[:, :, 0:T * H].rearrange(
                                "p b (t h) -> p b t h", h=H))
                        nc.vector.scalar_tensor_tensor(
                            lk[:], lk[:], NEG_SLOPE, lk[:], ALU.mult,
                            ALU.max)
                        lks[(w, r)] = lk

                def front_exp(w):
                    for r in range(2):
                        lk = lks.pop((w, r))
                        msgb = msgp.tile([128, B, 816], BF16,
                                         tag=f"msg{r}", bufs=1)
                        nc.scalar.activation(
                            msgb[:, :, 0:768].rearrange(
                                "p b (t h k) -> p b t h k", h=H, k=DH),
                            lk[:].rearrange("p b (t h) -> p b t h", h=H)
                            .unsqueeze(4).broadcast_to((128, B, T, H, DH)),
                            AF.Exp)
                        nc.scalar.activation(msgb[:, :, 768:816], lk[:],
                                             AF.Exp)
                        msgbs[(w, r)] = msgb

                prefetch_stb(0)
                prefetch_back(0)
                for w in range(min(PREPD, NW)):
                    prefetch_si(w)
                    issue_gather(w)
                for w in range(NW):
                    nw = _win_nodes(w)
                    if w + PREPD < NW:
                        prefetch_si(w + PREPD)
                        issue_gather(w + PREPD)
                    if w + 1 < NW:
                        prefetch_stb(w + 1)
                        prefetch_back(w + 1)
                    xcw, gb2, ssbs = pre_b.pop(w)
                    front_pre(w)
                    front_exp(w)
                    zg = zgs.pop(w)
                    msgbw = []
                    for r in range(2):
                        zap = zg[:, r * B:(r + 1) * B, 0:T * 68].rearrange(
                            "p b (t c) -> p b t c", c=68)[:, :, :, 0:64] \
                            .rearrange("p b t (h k) -> p b t h k", k=DH)
                        msgb = msgbs.pop((w, r))
                        mz = msgb[:, :, 0:768].rearrange(
                            "p b (t h k) -> p b t h k", h=H, k=DH)
                        nc.vector.tensor_mul(mz, mz, zap)
                        msgbw.append(msgb)
                    msum = []
                    for r in range(2):
                        msgb = msgbw[r]
                        ssb = ssbs[r]
                        ms = pp.tile([128, 816], F32, tag="big", bufs=2)
                        for b in range(B):
                            lhsT = ssb[:, b * 128:(b + 1) * 128]
                            nc.tensor.matmul(ms[:, 0:512], lhsT,
                                             msgb[:, b, 0:512],
                                             start=(b == 0),
                                             stop=(b == B - 1))
                            nc.tensor.matmul(ms[:, 512:816], lhsT,
                                             msgb[:, b, 512:816],
                                             start=(b == 0),
                                             stop=(b == B - 1))
                        msum.append(ms)
                    # epilogue: m = msgsum/denom; x2 = bf16(x + m1 + m2)
                    x2w = x2p.tile([128, T * D], BF16, tag="x2")
                    mtmp = p2t.tile([128, T * D], BF16, tag="mtmp")
                    for r in range(2):
                        rec = p2s.tile([128, T * H], F32, tag="rec")
                        nc.vector.tensor_scalar_max(
                            rec[:nw], msum[r][:nw, 768:816], 1e-16)
                        nc.vector.reciprocal(rec[:nw], rec[:nw])
                        rb = rec[:nw].rearrange(
                            "p (t h) -> p t h", h=H).unsqueeze(3) \
                            .broadcast_to((nw, T, H, DH))
                        dst = (mtmp if r == 0 else x2w)
                        nc.vector.tensor_mul(
                            dst[:nw].rearrange(
                                "p (t h k) -> p t h k", h=H, k=DH),
                            msum[r][:nw, 0:768].rearrange(
                                "p (t h k) -> p t h k", h=H, k=DH), rb)
                    nc.vector.tensor_add(mtmp[:nw], mtmp[:nw], xcw[:nw])
                    nc.vector.tensor_add(x2w[:nw], x2w[:nw], mtmp[:nw])
                    # BN2 stats + a2/b2
                    st6b = p2s.tile([128, 2, 6], F32, tag="st6b")
                    nc.vector.bn_stats(st6b[:nw, 0, :], x2w[:nw, 0:384])
                    nc.vector.bn_stats(st6b[:nw, 1, :], x2w[:nw, 384:768])
                    mvb = p2s.tile([128, 2], F32, tag="mvb")
                    nc.vector.bn_aggr(mvb[:nw], st6b[:nw])
                    rs2 = _emit_rsqrt(nc, p2s, mvb[:nw, 1:2], nw, 1, RC2,
                                      "b2")
                    ab2 = abp.tile([128, 2], F32, tag="ab2")
                    nc.vector.tensor_mul(ab2[:nw, 0:1], gb2[:nw, 0:1],
                                         rs2[:nw])
                    nc.vector.tensor_mul(ab2[:nw, 1:2], ab2[:nw, 0:1],
                                         mvb[:nw, 0:1])
                    nc.vector.tensor_sub(ab2[:nw, 1:2], gb2[:nw, 1:2],
                                         ab2[:nw, 1:2])
                    if phases < 3:
                        xo = p2t.tile([128, T * D], BF16, tag="mtmp")
                        nc.vector.tensor_copy(xo[:nw], x2w[:nw])
                        nc.sync.dma_start(out_d[w * WIN:w * WIN + nw],
                                          xo[:nw])
                        continue
                    # ---- FFN: BN2 apply + 2 layers + residual ----
                    h2 = p2t.tile([128, T * D], BF16, tag="hw")
                    nc.scalar.activation(h2[:nw], x2w[:nw], AF.Identity,
                                         bias=ab2[:nw, 1:2],
                                         scale=ab2[:nw, 0:1])
                    h2t = p2t.tile([64, T, 128], BF16, tag="h2t")
                    for half in range(2):
                        tp = pp.tile([64, NPAIR, 128], BF16, tag="tp3")
                        for j in range(NPAIR):
                            t = half * NPAIR + j
                            nc.tensor.transpose(
                                tp[:, j, 0:nw], h2[:nw, t * 64:(t + 1) * 64],
                                ident[:nw, :nw])
                        nc.scalar.activation(
                            h2t[:, half * NPAIR:(half + 1) * NPAIR, 0:nw],
                            tp[:, :, 0:nw], AF.Copy)
                    if nw < 128:
                        nc.vector.memset(h2t[:, :, nw:128], 0.0)
                    dd = pp.tile([128, T, 64], BF16, tag="tpS")
                    fft = p2t.tile([64, T, 128], BF16, tag="fft")
                    for k in range(3):
                        big = pp.tile([128, 816], F32, tag="big", bufs=2)
                        f1 = big[:, 0:512]
                        rhs = h2t[:, 4 * k:4 * k + 4, :]
                        nc.tensor.matmul(f1, ffw1[0:64, :],
                                         rhs.rearrange("p a b -> p (a b)"),
                                         start=True, stop=True)
                        g1 = p2t.tile([128, 512], BF16, tag="g1")
                        nc.scalar.activation(g1[:], f1, AF.Gelu,
                                             bias=ffb1[:])
                        for half in range(2):
                            f2 = big[0:64, 512:768]
                            nc.tensor.matmul(
                                f2, ffw2[:],
                                g1[:, half * 256:(half + 1) * 256],
                                start=True, stop=True)
                            nc.vector.tensor_scalar(
                                fft[:, 4 * k + 2 * half:
                                    4 * k + 2 * half + 2, :]
                                .rearrange("p a b -> p (a b)"),
                                f2, ffb2r[0:64, :], None, ALU.add)
                    for t in range(T):
                        nc.tensor.transpose(
                            dd[0:nw, t, :], fft[:, t, 0:nw],
                            ident[0:64, 0:64])
                    ot = p2t.tile([128, T * D], BF16, tag="mtmp")
                    nc.vector.tensor_add(
                        ot[:nw], dd[:nw].rearrange("p a b -> p (a b)"),
                        x2w[:nw])
                    nc.sync.dma_start(out_d[w * WIN:w * WIN + nw], ot[:nw])

    nc.compile()
    return nc


_CACHE = {}
_PHASES = 3
_TRACE = False
_TRACE_DIR = None
_LAST_EXEC_NS = None


def _host_prep(inputs):
    x = np.asarray(inputs["x"], np.float32)
    xf = np.ascontiguousarray(x.reshape(N, T * D))
    xbf_full = xf.astype(BF16NP)
    B = 0
    for r in (1, 2):
        B = max(B, _max_blocks(np.asarray(inputs[f"src{r}"]),
                               np.asarray(inputs[f"dst{r}"])))

    bn1_gb_full = np.ascontiguousarray(
        np.stack([np.asarray(inputs["bn1_g"], np.float32),
                  np.asarray(inputs["bn1_b"], np.float32)], axis=1))
    bn2_gb_full = np.ascontiguousarray(
        np.stack([np.asarray(inputs["bn2_g"], np.float32),
                  np.asarray(inputs["bn2_b"], np.float32)], axis=1))
    common = {
        "ffw1": np.ascontiguousarray(np.asarray(inputs["ff_w1"], np.float32)),
        "ffb1": np.ascontiguousarray(
            np.asarray(inputs["ff_b1"], np.float32).reshape(DFF, 1)),
        "ffw2": np.ascontiguousarray(np.asarray(inputs["ff_w2"], np.float32)),
        "ffb2": np.ascontiguousarray(
            np.asarray(inputs["ff_b2"], np.float32).reshape(D, 1)),
        "ident": np.eye(128, dtype=BF16NP),
    }
    for r in (1, 2):
        W = np.asarray(inputs[f"W{r}"], np.float32).reshape(D, H * DH)
        al = np.asarray(inputs[f"al{r}"], np.float32).reshape(-1)
        ar = np.asarray(inputs[f"ar{r}"], np.float32).reshape(-1)
        common[f"W{r}"] = np.ascontiguousarray(W)
        common[f"al{r}t"] = np.ascontiguousarray(np.tile(al[None, :], (D, 1)))
        common[f"ar{r}t"] = np.ascontiguousarray(np.tile(ar[None, :], (D, 1)))

    BL = B * 128
    in_maps = []
    for m in range(NCORES):
        lo = m * CHUNK
        im = dict(common)
        # rotated per-core node ordering: row i = node (lo + i) mod N
        im["x_bf"] = np.ascontiguousarray(np.roll(xbf_full, -lo, axis=0))
        im["bn1_gb"] = np.ascontiguousarray(np.roll(bn1_gb_full, -lo, axis=0))
        im["xc"] = np.ascontiguousarray(xbf_full[lo:lo + CHUNK])
        im["bn2_gb"] = np.ascontiguousarray(bn2_gb_full[lo:lo + CHUNK])
        srcs = []
        for r in (1, 2):
            src_flat, S, ST = _prep_core_rel(
                np.asarray(inputs[f"src{r}"]), np.asarray(inputs[f"dst{r}"]),
                lo, B)
            im[f"S{r}"] = S
            im[f"ST{r}"] = ST
            srcs.append(2 * src_flat + (r - 1))  # interleaved zpackB rows
        idx = np.empty((128, NW * (2 * BL) // 16), np.int16)
        for w in range(NW):
            for r in range(2):
                seg = srcs[r][w * BL:(w + 1) * BL].astype(np.int16)
                col0 = (w * 2 + r) * (BL // 16)
                idx[:, col0:col0 + BL // 16] = np.tile(
                    seg.reshape(-1, 16).T, (8, 1))
        im["srcidx"] = np.ascontiguousarray(idx)
        in_maps.append(im)
    return B, in_maps


def kernel(**inputs):
    B, in_maps = _host_prep(inputs)
    key = (B, _PHASES)
    if key not in _CACHE:
        _CACHE[key] = _build_program(B, _PHASES)
    nc = _CACHE[key]
    global _LAST_EXEC_NS
    tmpdir = None
    if _TRACE and _TRACE_DIR:
        import os, shutil
        shutil.rmtree(_TRACE_DIR, ignore_errors=True)
        os.makedirs(_TRACE_DIR, exist_ok=True)
        tmpdir = _TRACE_DIR
    res = run_bass_kernel_spmd(nc, in_maps, core_ids=list(range(NCORES)),
                               trace=_TRACE, tmpdir=tmpdir)
    _LAST_EXEC_NS = res.exec_time_ns
    out = np.concatenate([res.results[m]["OUT"] for m in range(NCORES)],
                         axis=0)
    return out.reshape(N, T, D).astype(np.float32)
